# revision 1
# baseline (speedup 1.0000x reference)
"""Trainium2 Bass kernel for nn_AdaptiveFusionNet (8-core data parallel).

Math collapse (validated vs reference to ~5e-6 rel):
  - DCT branch: apply_dct(x)[b,c] == A @ X @ A.T with A = R @ D[:,:25] @ D[:25,:]
    (R = 256->8 bilinear resize matrix; note reference's "inverse" einsum
    applies D again, not D.T).
  - Gradient branch: bilinear 256->8 resize samples grad magnitude only at
    rows/cols {15,16}+32g with weight 1/4 each; sobel there needs only the
    4x4 neighborhoods {14..17}+32g (wrap never triggered). So only 32x32
    samples per channel are needed.
  - conv+BN+ReLU folded: w_eff = w*gamma/sqrt(1+eps), b_eff = b*g+beta.
  - Final: out = w*Pd + (1-w)*Pg + cls_b computed on host from per-core
    [16,5] device outputs (Pd0,Pg0,Pd1,Pg1,slogit).

Per core (16 images, 48 image-channels):
  stage1 (PE):  W1T[ic][cb][k, b*8+i] = sum_r X[r, cb*128+k] * A[i, r]
                (192 f32 matmuls, lhsT = X blocks, N=8; per-image DMAs
                pipelined against it via a bufs=4 tile pool)
  stage2 (PE):  psum2[j, ic*128+b*8+i] = dct8[b,ic][i,j]
  transpose(PE) -> dt_sb[b*8+i, ic*8+j] -> 3 sbuf DMAs -> zero-padded
                dconv[ic, b, 10, 10] (padding implements conv borders,
                keeping every conv matmul a flat [64,512] output)
  grad (DMA+DVE+ACT): row-gather DMA (32 of 256 rows) -> col-sample
                (ScalarE copies) -> sobel at the 16x16 sampled points
                (DVE, per-lr 3-dim APs; deeper fused APs measured slower)
  conv (PE, f32r): 9-shift matmuls K=3(ic), N=512, both branches into
                disjoint PSUM partition-halves; f32r needs producers to
                emit f32r dtype (walrus checkMatmultFP32r)
  tail: relu (ACT, folded BN bias) -> per-batch-half reductions (DVE)
                -> 3 tiny matmuls -> out[16,5]; halves pipeline.
Final sigmoid-gated combine runs on host (cheap on [128,*]).

Measured (8 cores, axon): rel err 1.28e-3 vs reference; ~120us per
pipeline iteration loop-amortized incl ~50us For_i back-edge overhead
(single-shot estimated ~70-85us vs ~40us DMA roofline at 358 GB/s/core).
"""
import sys

import numpy as np

try:
    import concourse  # noqa: F401
except ImportError:
    sys.path.insert(0, "/opt/trn_rl_repo")

import concourse.bass as bass
import concourse.bacc as bacc
import concourse.mybir as mybir
from concourse import tile
from concourse.bass_utils import run_bass_kernel_spmd

F32 = mybir.dt.float32
F32R = mybir.dt.float32r
BF16 = mybir.dt.bfloat16
N_CORES = 8
B_TOTAL = 128
B = B_TOTAL // N_CORES  # 16 images per core
NCH = B * 3             # 48 channels per core


def build_A():
    N = 256
    n = np.arange(N, dtype=np.float64)
    k = n[:, None]
    D = np.cos(np.pi * (2.0 * n + 1.0) * k / (2.0 * N))
    scale = np.full((N, 1), np.sqrt(2.0 / N))
    scale[0, 0] = np.sqrt(1.0 / N)
    D = D * scale
    R = np.zeros((8, 256))
    for i in range(8):
        R[i, 15 + 32 * i] = 0.5
        R[i, 16 + 32 * i] = 0.5
    A = R @ D[:, :25] @ D[:25, :]
    return A.astype(np.float32)


def _build_nc(timing_loop=None):
    import os as _os
    drop = set(_os.environ.get("KDROP", "").split(","))
    nc = bacc.Bacc("TRN2", target_bir_lowering=False, debug=False,
                   num_devices=N_CORES)

    if timing_loop is None:
        x_d = nc.dram_tensor("x", [B, 3, 256, 256], F32, kind="ExternalInput")
    else:
        x_d = nc.dram_tensor("xint", [B, 3, 256, 256], F32)
    at_d = nc.dram_tensor("at", [256, 8], F32, kind="ExternalInput")
    id_d = nc.dram_tensor("ident", [8, 8], F32, kind="ExternalInput")
    convw_d = nc.dram_tensor("convw", [3, 1152], F32R, kind="ExternalInput")
    bias_d = nc.dram_tensor("bias", [128, 1], F32, kind="ExternalInput")
    ccls_d = nc.dram_tensor("ccls", [128, 128], F32, kind="ExternalInput")
    fusw_d = nc.dram_tensor("fusw", [128, 1], F32, kind="ExternalInput")
    sel_d = nc.dram_tensor("sel", [128, 2], F32, kind="ExternalInput")
    out_d = nc.dram_tensor("out", [B, 5], F32, kind="ExternalOutput")

    with tile.TileContext(nc) as tc:
        with (
            tc.tile_pool(name="const", bufs=1) as cpool,
            tc.tile_pool(name="xin", bufs=6) as xpool,
            tc.tile_pool(name="work", bufs=1) as wpool,
            tc.tile_pool(name="scratch", bufs=2) as spool,
            tc.tile_pool(name="ps1", bufs=1, space="PSUM") as ps1,
            tc.tile_pool(name="ps2", bufs=1, space="PSUM") as ps2,
        ):
            # ---- constants ----
            a1t = cpool.tile([128, 8], F32, tag="a1t")
            a2t = cpool.tile([128, 8], F32, tag="a2t")
            nc.sync.dma_start(a1t[:], at_d[0:128, :])
            nc.sync.dma_start(a2t[:], at_d[128:256, :])
            ident = cpool.tile([8, 8], F32, tag="ident")
            nc.sync.dma_start(ident[:], id_d[:])
            convw = cpool.tile([3, 1152], F32R, tag="convw")
            nc.sync.dma_start(convw[:], convw_d[:])
            biasd = cpool.tile([64, 1], F32, tag="biasd")
            biasg = cpool.tile([64, 1], F32, tag="biasg")
            nc.sync.dma_start(biasd[:], bias_d[0:64, :])
            nc.sync.dma_start(biasg[:], bias_d[64:128, :])
            zpad = cpool.tile([3, 160], F32, tag="zpad")
            nc.vector.memset(zpad[:], 0.0)
            ccls = cpool.tile([128, 128], F32, tag="ccls")
            nc.sync.dma_start(ccls[:], ccls_d[:])
            fusw = cpool.tile([128, 1], F32, tag="fusw")
            nc.sync.dma_start(fusw[:], fusw_d[:])
            sel = cpool.tile([128, 2], F32, tag="sel")
            nc.sync.dma_start(sel[:], sel_d[:])

            def emit():
                # ---- stage 1: X^T A^T per channel ----
                w1t_ps = [[ps1.tile([128, 128], F32, tag=f"w1t_{ic}_{cb}",
                                    name=f"w1t_{ic}_{cb}")
                           for cb in range(2)] for ic in range(3)]

                def w1t_view(ic, cb):
                    return w1t_ps[ic][cb][:]
                for b in range(B):
                    xt = xpool.tile([128, 3, 2, 256], F32, tag="xt")
                    nc.sync.dma_start(
                        xt[:], x_d[b].rearrange("c (rb p) w -> p c rb w", p=128))
                    for ic in range(3):
                        for cb in range(2):
                            for rb in range(2):
                                nc.tensor.matmul(
                                    w1t_view(ic, cb)[:, b * 8:(b + 1) * 8],
                                    lhsT=xt[:, ic, rb, cb * 128:(cb + 1) * 128],
                                    rhs=(a1t[:] if rb == 0 else a2t[:]),
                                    start=(rb == 0), stop=(rb == 1))

                # ---- gradient branch: row gather DMAs (ic-major partitions) ----
                gp = wpool.tile([NCH, 8, 4, 256], F32, tag="gp")
                if "gpdma" not in drop:
                    for ic in range(3):
                        src = x_d[:, ic].rearrange(
                            "b (g h) w -> b g h w", h=32)[:, :, 14:18, :]
                        nc.sync.dma_start(gp[ic * B:(ic + 1) * B], src)

                # ---- stage 2: dct8^T ----
                dcttail = "dcttail" not in drop
                w1t_sb = [[wpool.tile([128, 128], F32, tag=f"w1sb_{ic}_{cb}", name=f"w1sb_{ic}_{cb}")
                           for cb in range(2)] for ic in range(3)]
                if dcttail:
                    for ic in range(3):
                        for cb in range(2):
                            nc.vector.tensor_copy(w1t_sb[ic][cb][:],
                                                  w1t_view(ic, cb))
                psum2_t = ps2.tile([8, 384], F32, tag="psA", name="psum2")
                psum2 = psum2_t[:]
                psumT_t = ps2.tile([128, 24], F32, tag="psB", name="psumT")
                psumT = psumT_t[:]
                if dcttail:
                    for ic in range(3):
                        for cb in range(2):
                            nc.tensor.matmul(
                                psum2[:, ic * 128:(ic + 1) * 128],
                                lhsT=(a1t[:] if cb == 0 else a2t[:]),
                                rhs=w1t_sb[ic][cb][:],
                                start=(cb == 0), stop=(cb == 1))

                dsb = wpool.tile([8, 384], F32, tag="dsb")
                dt_sb = wpool.tile([128, 24], F32, tag="dt_sb")
                dflat = wpool.tile([3, 1024], F32, tag="dflat")
                dconv = wpool.tile([3, B, 10, 10], F32R, tag="dconv")

                def zero_borders(pad_t):
                    z10 = zpad.rearrange("p (b w) -> p b w", b=B)          # [3,16,10]
                    z8 = zpad[:, 0:128].rearrange("p (b w) -> p b w", b=B)  # [3,16,8]
                    nc.vector.tensor_copy(pad_t[:, :, 0, :], z10)
                    nc.vector.tensor_copy(pad_t[:, :, 9, :], z10)
                    nc.vector.tensor_copy(pad_t[:, :, 1:9, 0], z8)
                    nc.vector.tensor_copy(pad_t[:, :, 1:9, 9], z8)

                if not dcttail:
                    zero_borders(dconv)
                if dcttail:
                    nc.vector.tensor_copy(dsb[:], psum2)
                    for ic in range(3):
                        nc.tensor.transpose(
                            psumT[:, ic * 8:(ic + 1) * 8],
                            dsb[0:8, ic * 128:(ic + 1) * 128],
                            ident[:])
                    nc.vector.tensor_copy(dt_sb[:], psumT)
                    for ic in range(3):
                        nc.sync.dma_start(dflat[ic:ic + 1, :],
                                          dt_sb[:, ic * 8:(ic + 1) * 8])
                    zero_borders(dconv)
                    nc.vector.tensor_copy(
                        dconv[:, :, 1:9, 1:9],
                        dflat.rearrange("c (b i j) -> c b i j", b=B, i=8))

                # ---- gradient branch: col sample + sobel ----
                gpatch = wpool.tile([NCH, 8, 4, 8, 4], F32, tag="gpatch")
                g8 = wpool.tile([NCH, 8, 8, 1], F32, tag="g8")
                gflat = wpool.tile([3, 1024], F32, tag="gflat")
                gconv = wpool.tile([3, B, 10, 10], F32R, tag="gconv")
                SUB = mybir.AluOpType.subtract
                ADD = mybir.AluOpType.add
                MUL = mybir.AluOpType.mult
                if "sobel" in drop:
                    zero_borders(gconv)
                if "sobel" not in drop:
                    gp5 = gp.rearrange("p g r (gc c) -> p g r gc c", c=32)
                    for g in range(8):
                        nc.scalar.copy(gpatch[:, g], gp5[:, g, :, :, 14:18])

                    def PP(r, dc):
                        return gpatch[:, :, r, :, 1 + dc:3 + dc]  # [48, 8, 8, 2]

                    SUB = mybir.AluOpType.subtract
                    ADD = mybir.AluOpType.add
                    MUL = mybir.AluOpType.mult
                    a_lr = []
                    for lr in (1, 2):
                        t1 = spool.tile([NCH, 8, 8, 2], F32, tag="t1")
                        t2 = spool.tile([NCH, 8, 8, 2], F32, tag="t2")
                        t3 = spool.tile([NCH, 8, 8, 2], F32, tag="t3")
                        nc.vector.tensor_tensor(t1[:], PP(lr - 1, 1), PP(lr - 1, -1), SUB)
                        nc.vector.tensor_tensor(t2[:], PP(lr, 1), PP(lr, -1), SUB)
                        nc.vector.tensor_tensor(t3[:], PP(lr + 1, 1), PP(lr + 1, -1), SUB)
                        u = spool.tile([NCH, 8, 8, 2], F32, tag="u")
                        nc.vector.tensor_tensor(u[:], t1[:], t3[:], ADD)
                        gx = spool.tile([NCH, 8, 8, 2], F32, tag="gx")
                        nc.vector.scalar_tensor_tensor(gx[:], t2[:], 2.0, u[:], MUL, ADD)
                        s1 = spool.tile([NCH, 8, 8, 2], F32, tag="s1")
                        s2 = spool.tile([NCH, 8, 8, 2], F32, tag="s2")
                        s3 = spool.tile([NCH, 8, 8, 2], F32, tag="s3")
                        nc.vector.tensor_tensor(s1[:], PP(lr + 1, -1), PP(lr - 1, -1), SUB)
                        nc.vector.tensor_tensor(s2[:], PP(lr + 1, 0), PP(lr - 1, 0), SUB)
                        nc.vector.tensor_tensor(s3[:], PP(lr + 1, 1), PP(lr - 1, 1), SUB)
                        u2 = spool.tile([NCH, 8, 8, 2], F32, tag="u2")
                        nc.vector.tensor_tensor(u2[:], s1[:], s3[:], ADD)
                        gy = spool.tile([NCH, 8, 8, 2], F32, tag="gy")
                        nc.vector.scalar_tensor_tensor(gy[:], s2[:], 2.0, u2[:], MUL, ADD)
                        nc.vector.tensor_tensor(gx[:], gx[:], gx[:], MUL)
                        nc.vector.tensor_tensor(gy[:], gy[:], gy[:], MUL)
                        m2 = spool.tile([NCH, 8, 8, 2], F32, tag="m2")
                        nc.vector.tensor_tensor(m2[:], gx[:], gy[:], ADD)
                        mag = spool.tile([NCH, 8, 8, 2], F32, tag=f"mag{lr}")
                        nc.scalar.sqrt(mag[:], m2[:])
                        al = spool.tile([NCH, 8, 8, 1], F32, tag=f"al{lr}")
                        nc.vector.tensor_tensor(al[:], mag[:, :, :, 0:1], mag[:, :, :, 1:2], ADD)
                        a_lr.append(al)

                    nc.vector.tensor_tensor(g8[:], a_lr[0][:], a_lr[1][:], ADD)
                    g8f = g8.rearrange("p a b c -> p (a b c)")  # [48, 64]

                    for ic in range(3):
                        nc.sync.dma_start(gflat[ic:ic + 1, :],
                                          g8f[ic * B:(ic + 1) * B, :])
                    zero_borders(gconv)
                    nc.vector.tensor_copy(
                        gconv[:, :, 1:9, 1:9],
                        gflat.rearrange("c (b i j) -> c b i j", b=B, i=8))


                # ---- convs (f32r via bitcast) ----
                psc = [[ps1.tile([64, 512], F32, tag=f"w1t_{br}_{nh}",
                                 name=f"psc_{br}_{nh}") for nh in range(2)]
                       for br in range(2)]
                if not ("conv" in drop or "tail" in drop):
                    for nh in range(2):
                        for br, rhs_t in ((0, dconv), (1, gconv)):
                            for si, (di, dj) in enumerate(
                                    (di, dj) for di in range(3) for dj in range(3)):
                                w_off = (br * 9 + di * 3 + dj) * 64
                                rv = rhs_t[:, nh * 8:(nh + 1) * 8,
                                           di:di + 8, dj:dj + 8]
                                nc.tensor.matmul(
                                    psc[br][nh][:, :],
                                    lhsT=convw[:, w_off:w_off + 64],
                                    rhs=rv,
                                    start=(si == 0), stop=(si == 8))

                # relu(conv + bias) -> dg_sb [128, 1024]
                tail_on = "tail" not in drop
                dg_sb = wpool.tile([128, 1024], F32, tag="dg_sb")
                if tail_on:
                    for nh in range(2):
                        for br in range(2):
                            nc.scalar.activation(
                                dg_sb[br * 64:(br + 1) * 64,
                                      nh * 512:(nh + 1) * 512],
                                psc[br][nh][:],
                                mybir.ActivationFunctionType.Relu,
                                bias=(biasd[:] if br == 0 else biasg[:]),
                                scale=1.0)

                    # ---- fusion + classifier (per half for pipelining) ----
                    HB = B // 2
                    psum_out = []
                    for nh in range(2):
                        dgh = dg_sb[:, nh * 512:(nh + 1) * 512].rearrange(
                            "p (b f) -> p b f", b=HB)
                        s_red = wpool.tile([128, HB, 1], F32, tag=f"s_red{nh}",
                                           name=f"s_red{nh}")
                        nc.vector.reduce_sum(s_red[:], dgh,
                                             axis=mybir.AxisListType.X)
                        tk_red = []
                        for k in range(2):
                            tmpk = spool.tile([128, HB, 64], F32, tag="tmpk")
                            cc = ccls[:, k * 64:(k + 1) * 64].unsqueeze(1)
                            nc.vector.tensor_tensor(
                                tmpk[:], dgh, cc.broadcast_to([128, HB, 64]), MUL)
                            tkr = wpool.tile([128, HB, 1], F32,
                                             tag=f"tkr_{nh}_{k}",
                                             name=f"tkr_{nh}_{k}")
                            nc.vector.reduce_sum(tkr[:], tmpk[:],
                                                 axis=mybir.AxisListType.X)
                            tk_red.append(tkr)
                        po = ps2.tile([HB, 5], F32,
                                      tag=("psA" if nh == 0 else "psB"),
                                      name=f"psum_out{nh}")
                        for k in range(2):
                            nc.tensor.matmul(po[0:HB, 2 * k:2 * k + 2],
                                             lhsT=tk_red[k][:], rhs=sel[:],
                                             start=True, stop=True)
                        nc.tensor.matmul(po[0:HB, 4:5], lhsT=s_red[:],
                                         rhs=fusw[:], start=True, stop=True)
                        psum_out.append(po)

                if tail_on:
                    for nh in range(2):
                        osb = wpool.tile([8, 5], F32, tag=f"osb{nh}",
                                         name=f"osb{nh}")
                        nc.vector.tensor_copy(osb[:], psum_out[nh][:])
                        nc.sync.dma_start(out_d[nh * 8:(nh + 1) * 8, :],
                                          osb[:])
                else:
                    out_sb = wpool.tile([16, 5], F32, tag="out_sb")
                    nc.vector.memset(out_sb[:], 0.0)
                    nc.sync.dma_start(out_d[:], out_sb[:])

            if timing_loop is None:
                emit()
            else:
                krep = int(_os.environ.get("KREP", "1"))
                kw = {}
                if _os.environ.get("KSTAG", "0") == "1":
                    kw["staggered_reset"] = True
                if _os.environ.get("KHINT", "0") == "1":
                    kw["hint_engines"] = (mybir.EngineType.PE,
                                          mybir.EngineType.SP,
                                          mybir.EngineType.DVE,
                                          mybir.EngineType.Activation)
                with tc.For_i(0, timing_loop, 1, **kw):
                    for _ in range(krep):
                        emit()

    nc.compile()
    return nc


_NC = {}


def _get_nc(timing_loop=None):
    if timing_loop not in _NC:
        _NC[timing_loop] = _build_nc(timing_loop)
    return _NC[timing_loop]


def _make_consts(conv_dct_w, conv_dct_b, bn_dct_g, bn_dct_b,
                 conv_grad_w, conv_grad_b, bn_grad_g, bn_grad_b,
                 fus_w, cls_w):
    A = build_A()
    consts = {}
    consts["at"] = np.ascontiguousarray(A.T)
    consts["ident"] = np.eye(8, dtype=np.float32)

    BN_EPS = 1e-5
    convw = np.zeros((3, 1152), np.float32)
    bias = np.zeros((128, 1), np.float32)
    for br, (w, b, g, beta) in enumerate((
            (conv_dct_w, conv_dct_b, bn_dct_g, bn_dct_b),
            (conv_grad_w, conv_grad_b, bn_grad_g, bn_grad_b))):
        g_eff = (g / np.sqrt(1.0 + BN_EPS)).astype(np.float32)
        w_eff = w * g_eff[:, None, None, None]
        if br == 1:
            w_eff = w_eff * 0.25  # fold the 4-sample average
        b_eff = b * g_eff + beta
        for di in range(3):
            for dj in range(3):
                # convw[ic, (br*9+di*3+dj)*64 + oc] = w_eff[oc, ic, di, dj]
                off = (br * 9 + di * 3 + dj) * 64
                convw[:, off:off + 64] = w_eff[:, :, di, dj].T
        bias[br * 64:(br + 1) * 64, 0] = b_eff
    consts["convw"] = convw
    consts["bias"] = bias

    ccls = np.zeros((128, 128), np.float32)
    for k in range(2):
        ccls[0:64, k * 64:(k + 1) * 64] = cls_w[k].reshape(64, 64)
        ccls[64:128, k * 64:(k + 1) * 64] = cls_w[k].reshape(64, 64)
    consts["ccls"] = ccls
    consts["fusw"] = np.ascontiguousarray(np.tile(fus_w[0][:, None] / 64.0, (2, 1)))
    sel = np.zeros((128, 2), np.float32)
    sel[0:64, 0] = 1.0
    sel[64:128, 1] = 1.0
    consts["sel"] = sel
    return consts


def kernel_with_results(x, conv_dct_w, conv_dct_b, bn_dct_g, bn_dct_b,
                        conv_grad_w, conv_grad_b, bn_grad_g, bn_grad_b,
                        fus_w, fus_b, cls_w, cls_b, trace=False):
    nc = _get_nc()
    consts = _make_consts(conv_dct_w, conv_dct_b, bn_dct_g, bn_dct_b,
                          conv_grad_w, conv_grad_b, bn_grad_g, bn_grad_b,
                          fus_w, cls_w)
    x = np.ascontiguousarray(np.asarray(x, np.float32))
    in_maps = []
    for i in range(N_CORES):
        m = {"x": np.ascontiguousarray(x[i * B:(i + 1) * B])}
        m.update(consts)
        in_maps.append(m)
    res = run_bass_kernel_spmd(nc, in_maps, list(range(N_CORES)), trace=trace)

    outs = []
    for i in range(N_CORES):
        r = res.results[i]["out"]  # [16, 5]
        Pd = r[:, [0, 2]]
        Pg = r[:, [1, 3]]
        sl = r[:, 4] + np.float32(fus_b[0])
        w = 1.0 / (1.0 + np.exp(-sl))[:, None]
        outs.append(w * Pd + (1.0 - w) * Pg + np.asarray(cls_b)[None, :])
    return np.concatenate(outs, axis=0).astype(np.float32), res


def kernel(**inputs):
    out, _ = kernel_with_results(**inputs)
    return out



# revision 4
# speedup vs baseline: 1.5239x; 1.5239x over previous
"""Trainium2 Bass kernel for nn_AdaptiveFusionNet (8-core data parallel).

Math collapse (validated vs reference):
  - DCT branch: apply_dct(x)[b,c] == A @ X @ A.T with A = R @ D[:,:25] @ D[:25,:]
    (R = 256->8 bilinear resize matrix; note reference's "inverse" einsum
    applies D again, not D.T).
  - Gradient branch: bilinear 256->8 resize samples grad magnitude only at
    rows/cols {15,16}+32g with weight 1/4 each; sobel there needs only the
    4x4 neighborhoods {14..17}+32g (wrap never triggered). So only 32x32
    samples per channel are needed.
  - conv+BN+ReLU folded: w_eff = w*gamma/sqrt(1+eps), b_eff = b*g+beta.
  - Final: out = w*Pd + (1-w)*Pg + cls_b computed on host from per-core
    [16,5] device outputs (Pd0,Pg0,Pd1,Pg1,slogit).

v2: DCT pipeline in bf16. Trace analysis of v1 showed PE-bound: fp32
matmuls lower to 2 HW passes with ~205ns 128-col weight loads each
(stage1 = 192 fp32 MMs = ~82us of PE). bf16 enables FWL (fast weight
load) and single-pass MMs. x is DMAed in f32 (HWDGE) then cast to bf16
per image, alternating DVE/ACT so neither engine bottlenecks. Sobel
still computes f32 (gp gather stays f32) but emits bf16 for the conv.

Per core (16 images, 48 image-channels):
  stage1 (PE):  W1T[ic][cb][k, b*8+i] = sum_r Xb[r, cb*128+k] * A[i, r]
                (192 bf16 matmuls, lhsT = cast X blocks, N=8)
  stage2 (PE):  psum2[j, ic*128+b*8+i] = dct8[b,ic][i,j]
  transpose(PE) -> dt_sb[b*8+i, ic*8+j] -> 3 sbuf DMAs -> zero-padded
                dconv[ic, b, 10, 10] (bf16)
  grad (DMA+DVE+ACT): row-gather DMA (32 of 256 rows) -> col-sample
                (ScalarE copies) -> sobel at the 16x16 sampled points
  conv (PE, bf16): 9-shift matmuls K=3(ic), N=512, both branches into
                disjoint PSUM partition-halves
  tail: relu (ACT, folded BN bias) -> bf16 -> per-batch-half reductions
                (DVE) -> 3 tiny matmuls -> out[16,5]; halves pipeline.
Final sigmoid-gated combine runs on host (cheap on [128,*]).
"""
import sys

import numpy as np

try:
    import concourse  # noqa: F401
except ImportError:
    sys.path.insert(0, "/opt/trn_rl_repo")

import concourse.bass as bass
import concourse.bacc as bacc
import concourse.mybir as mybir
from concourse import tile
from concourse.bass_utils import run_bass_kernel_spmd

F32 = mybir.dt.float32
import os as _os_dt
DT16 = (mybir.dt.float16 if _os_dt.environ.get("KDT16", "fp16") == "fp16"
        else mybir.dt.bfloat16)
NP_DT16 = mybir.dt.np(DT16)
N_CORES = 8
B_TOTAL = 128
B = B_TOTAL // N_CORES  # 16 images per core
NCH = B * 3             # 48 channels per core


def build_A():
    N = 256
    n = np.arange(N, dtype=np.float64)
    k = n[:, None]
    D = np.cos(np.pi * (2.0 * n + 1.0) * k / (2.0 * N))
    scale = np.full((N, 1), np.sqrt(2.0 / N))
    scale[0, 0] = np.sqrt(1.0 / N)
    D = D * scale
    R = np.zeros((8, 256))
    for i in range(8):
        R[i, 15 + 32 * i] = 0.5
        R[i, 16 + 32 * i] = 0.5
    A = R @ D[:, :25] @ D[:25, :]
    return A.astype(np.float32)


def _build_nc(timing_loop=None):
    import os as _os
    drop = set(_os.environ.get("KDROP", "").split(","))
    nc = bacc.Bacc("TRN2", target_bir_lowering=False, debug=False,
                   num_devices=N_CORES)

    if timing_loop is None:
        x_d = nc.dram_tensor("x", [B, 3, 256, 256], F32, kind="ExternalInput")
    else:
        x_d = nc.dram_tensor("xint", [B, 3, 256, 256], F32)
    at_d = nc.dram_tensor("at", [256, 8], DT16, kind="ExternalInput")
    id_d = nc.dram_tensor("ident", [8, 8], DT16, kind="ExternalInput")
    convw_d = nc.dram_tensor("convw", [3, 1152], DT16, kind="ExternalInput")
    bias_d = nc.dram_tensor("bias", [128, 1], F32, kind="ExternalInput")
    ccls_d = nc.dram_tensor("ccls", [128, 128], DT16, kind="ExternalInput")
    fusw_d = nc.dram_tensor("fusw", [128, 1], F32, kind="ExternalInput")
    sel_d = nc.dram_tensor("sel", [128, 2], F32, kind="ExternalInput")
    out_d = nc.dram_tensor("out", [B, 5], F32, kind="ExternalOutput")

    with tile.TileContext(nc) as tc:
        with (
            tc.tile_pool(name="const", bufs=1) as cpool,
            tc.tile_pool(name="xin", bufs=6) as xpool,
            tc.tile_pool(name="xbf", bufs=4) as xbpool,
            tc.tile_pool(name="work", bufs=1) as wpool,
            tc.tile_pool(name="scratch", bufs=2) as spool,
            tc.tile_pool(name="ps1", bufs=1, space="PSUM") as ps1,
            tc.tile_pool(name="ps2", bufs=1, space="PSUM") as ps2,
        ):
            # ---- constants ----
            a1t = cpool.tile([128, 8], DT16, tag="a1t")
            a2t = cpool.tile([128, 8], DT16, tag="a2t")
            nc.sync.dma_start(a1t[:], at_d[0:128, :])
            nc.sync.dma_start(a2t[:], at_d[128:256, :])
            ident = cpool.tile([8, 8], DT16, tag="ident")
            nc.sync.dma_start(ident[:], id_d[:])
            convw = cpool.tile([3, 1152], DT16, tag="convw")
            nc.sync.dma_start(convw[:], convw_d[:])
            biasd = cpool.tile([64, 1], F32, tag="biasd")
            biasg = cpool.tile([64, 1], F32, tag="biasg")
            nc.sync.dma_start(biasd[:], bias_d[0:64, :])
            nc.sync.dma_start(biasg[:], bias_d[64:128, :])
            zpad = cpool.tile([3, 160], DT16, tag="zpad")
            nc.vector.memset(zpad[:], 0.0)
            ccls = cpool.tile([128, 128], DT16, tag="ccls")
            nc.sync.dma_start(ccls[:], ccls_d[:])
            fusw = cpool.tile([128, 1], F32, tag="fusw")
            nc.sync.dma_start(fusw[:], fusw_d[:])
            sel = cpool.tile([128, 2], F32, tag="sel")
            nc.sync.dma_start(sel[:], sel_d[:])

            def emit():
                # ---- stage 1: X^T A^T per channel (bf16) ----
                w1t_ps = [[ps1.tile([128, 128], F32, tag=f"w1t_{ic}_{cb}",
                                    name=f"w1t_{ic}_{cb}")
                           for cb in range(2)] for ic in range(3)]

                def w1t_view(ic, cb):
                    return w1t_ps[ic][cb][:]
                for b in range(B):
                    xt = xpool.tile([128, 3, 2, 256], F32, tag="xt")
                    nc.sync.dma_start(
                        xt[:], x_d[b].rearrange("c (rb p) w -> p c rb w", p=128))
                    xtb = xbpool.tile([128, 3, 2, 256], DT16, tag="xtb")
                    if b % 2 == 0:
                        nc.vector.tensor_copy(xtb[:], xt[:])
                    else:
                        nc.scalar.copy(xtb[:], xt[:])
                    for ic in range(3):
                        for cb in range(2):
                            for rb in range(2):
                                nc.tensor.matmul(
                                    w1t_view(ic, cb)[:, b * 8:(b + 1) * 8],
                                    lhsT=xtb[:, ic, rb, cb * 128:(cb + 1) * 128],
                                    rhs=(a1t[:] if rb == 0 else a2t[:]),
                                    start=(rb == 0), stop=(rb == 1))

                # ---- gradient branch: row gather DMAs (ic-major partitions) ----
                gp = wpool.tile([NCH, 8, 4, 256], F32, tag="gp")
                if "gpdma" not in drop:
                    for ic in range(3):
                        src = x_d[:, ic].rearrange(
                            "b (g h) w -> b g h w", h=32)[:, :, 14:18, :]
                        nc.sync.dma_start(gp[ic * B:(ic + 1) * B], src)

                # ---- stage 2: dct8^T ----
                dcttail = "dcttail" not in drop
                w1t_sb = [[wpool.tile([128, 128], DT16, tag=f"w1sb_{ic}_{cb}",
                                      name=f"w1sb_{ic}_{cb}")
                           for cb in range(2)] for ic in range(3)]
                if dcttail:
                    for ic in range(3):
                        for cb in range(2):
                            nc.vector.tensor_copy(w1t_sb[ic][cb][:],
                                                  w1t_view(ic, cb))
                psum2_t = ps2.tile([8, 384], F32, tag="psA", name="psum2")
                psum2 = psum2_t[:]
                psumT_t = ps2.tile([128, 24], DT16, tag="psB", name="psumT")
                psumT = psumT_t[:]
                if dcttail:
                    for ic in range(3):
                        for cb in range(2):
                            nc.tensor.matmul(
                                psum2[:, ic * 128:(ic + 1) * 128],
                                lhsT=(a1t[:] if cb == 0 else a2t[:]),
                                rhs=w1t_sb[ic][cb][:],
                                start=(cb == 0), stop=(cb == 1))

                dsb = wpool.tile([8, 384], DT16, tag="dsb")
                dt_sb = wpool.tile([128, 24], DT16, tag="dt_sb")
                dflat = wpool.tile([3, 1024], DT16, tag="dflat")
                dconv = wpool.tile([3, B, 10, 10], DT16, tag="dconv")

                def zero_borders(pad_t):
                    z10 = zpad.rearrange("p (b w) -> p b w", b=B)          # [3,16,10]
                    z8 = zpad[:, 0:128].rearrange("p (b w) -> p b w", b=B)  # [3,16,8]
                    nc.vector.tensor_copy(pad_t[:, :, 0, :], z10)
                    nc.vector.tensor_copy(pad_t[:, :, 9, :], z10)
                    nc.vector.tensor_copy(pad_t[:, :, 1:9, 0], z8)
                    nc.vector.tensor_copy(pad_t[:, :, 1:9, 9], z8)

                if not dcttail:
                    zero_borders(dconv)
                if dcttail:
                    nc.vector.tensor_copy(dsb[:], psum2)
                    for ic in range(3):
                        nc.tensor.transpose(
                            psumT[:, ic * 8:(ic + 1) * 8],
                            dsb[0:8, ic * 128:(ic + 1) * 128],
                            ident[:])
                    nc.vector.tensor_copy(dt_sb[:], psumT)
                    for ic in range(3):
                        nc.sync.dma_start(dflat[ic:ic + 1, :],
                                          dt_sb[:, ic * 8:(ic + 1) * 8])
                    zero_borders(dconv)
                    nc.vector.tensor_copy(
                        dconv[:, :, 1:9, 1:9],
                        dflat.rearrange("c (b i j) -> c b i j", b=B, i=8))

                # ---- gradient branch: col sample + sobel (f32 in, bf16 out) ----
                gpatch = wpool.tile([NCH, 8, 4, 8, 4], F32, tag="gpatch")
                g8 = wpool.tile([NCH, 8, 8, 1], DT16, tag="g8")
                gflat = wpool.tile([3, 1024], DT16, tag="gflat")
                gconv = wpool.tile([3, B, 10, 10], DT16, tag="gconv")
                SUB = mybir.AluOpType.subtract
                ADD = mybir.AluOpType.add
                MUL = mybir.AluOpType.mult
                if "sobel" in drop:
                    zero_borders(gconv)
                if "sobel" not in drop:
                    gp5 = gp.rearrange("p g r (gc c) -> p g r gc c", c=32)
                    for g in range(8):
                        nc.scalar.copy(gpatch[:, g], gp5[:, g, :, :, 14:18])

                    def PP(r, dc):
                        return gpatch[:, :, r, :, 1 + dc:3 + dc]  # [48, 8, 8, 2]

                    a_lr = []
                    for lr in (1, 2):
                        t1 = spool.tile([NCH, 8, 8, 2], F32, tag="t1")
                        t2 = spool.tile([NCH, 8, 8, 2], F32, tag="t2")
                        t3 = spool.tile([NCH, 8, 8, 2], F32, tag="t3")
                        nc.vector.tensor_tensor(t1[:], PP(lr - 1, 1), PP(lr - 1, -1), SUB)
                        nc.vector.tensor_tensor(t2[:], PP(lr, 1), PP(lr, -1), SUB)
                        nc.vector.tensor_tensor(t3[:], PP(lr + 1, 1), PP(lr + 1, -1), SUB)
                        u = spool.tile([NCH, 8, 8, 2], F32, tag="u")
                        nc.vector.tensor_tensor(u[:], t1[:], t3[:], ADD)
                        gx = spool.tile([NCH, 8, 8, 2], F32, tag="gx")
                        nc.vector.scalar_tensor_tensor(gx[:], t2[:], 2.0, u[:], MUL, ADD)
                        s1 = spool.tile([NCH, 8, 8, 2], F32, tag="s1")
                        s2 = spool.tile([NCH, 8, 8, 2], F32, tag="s2")
                        s3 = spool.tile([NCH, 8, 8, 2], F32, tag="s3")
                        nc.vector.tensor_tensor(s1[:], PP(lr + 1, -1), PP(lr - 1, -1), SUB)
                        nc.vector.tensor_tensor(s2[:], PP(lr + 1, 0), PP(lr - 1, 0), SUB)
                        nc.vector.tensor_tensor(s3[:], PP(lr + 1, 1), PP(lr - 1, 1), SUB)
                        u2 = spool.tile([NCH, 8, 8, 2], F32, tag="u2")
                        nc.vector.tensor_tensor(u2[:], s1[:], s3[:], ADD)
                        gy = spool.tile([NCH, 8, 8, 2], F32, tag="gy")
                        nc.vector.scalar_tensor_tensor(gy[:], s2[:], 2.0, u2[:], MUL, ADD)
                        nc.vector.tensor_tensor(gx[:], gx[:], gx[:], MUL)
                        nc.vector.tensor_tensor(gy[:], gy[:], gy[:], MUL)
                        m2 = spool.tile([NCH, 8, 8, 2], F32, tag="m2")
                        nc.vector.tensor_tensor(m2[:], gx[:], gy[:], ADD)
                        mag = spool.tile([NCH, 8, 8, 2], F32, tag=f"mag{lr}")
                        nc.scalar.sqrt(mag[:], m2[:])
                        al = spool.tile([NCH, 8, 8, 1], F32, tag=f"al{lr}")
                        nc.vector.tensor_tensor(al[:], mag[:, :, :, 0:1], mag[:, :, :, 1:2], ADD)
                        a_lr.append(al)

                    nc.vector.tensor_tensor(g8[:], a_lr[0][:], a_lr[1][:], ADD)
                    g8f = g8.rearrange("p a b c -> p (a b c)")  # [48, 64]

                    for ic in range(3):
                        nc.sync.dma_start(gflat[ic:ic + 1, :],
                                          g8f[ic * B:(ic + 1) * B, :])
                    zero_borders(gconv)
                    nc.vector.tensor_copy(
                        gconv[:, :, 1:9, 1:9],
                        gflat.rearrange("c (b i j) -> c b i j", b=B, i=8))

                # ---- convs (bf16) ----
                psc = [[ps1.tile([64, 512], F32, tag=f"w1t_{br}_{nh}",
                                 name=f"psc_{br}_{nh}") for nh in range(2)]
                       for br in range(2)]
                if not ("conv" in drop or "tail" in drop):
                    for nh in range(2):
                        for br, rhs_t in ((0, dconv), (1, gconv)):
                            for si, (di, dj) in enumerate(
                                    (di, dj) for di in range(3) for dj in range(3)):
                                w_off = (br * 9 + di * 3 + dj) * 64
                                rv = rhs_t[:, nh * 8:(nh + 1) * 8,
                                           di:di + 8, dj:dj + 8]
                                nc.tensor.matmul(
                                    psc[br][nh][:, :],
                                    lhsT=convw[:, w_off:w_off + 64],
                                    rhs=rv,
                                    start=(si == 0), stop=(si == 8))

                # relu(conv + bias) -> dg_sb [128, 1024] bf16
                tail_on = "tail" not in drop
                dg_sb = wpool.tile([128, 1024], DT16, tag="dg_sb")
                if tail_on:
                    for nh in range(2):
                        for br in range(2):
                            nc.scalar.activation(
                                dg_sb[br * 64:(br + 1) * 64,
                                      nh * 512:(nh + 1) * 512],
                                psc[br][nh][:],
                                mybir.ActivationFunctionType.Relu,
                                bias=(biasd[:] if br == 0 else biasg[:]),
                                scale=1.0)

                    # ---- fusion + classifier (per half for pipelining) ----
                    HB = B // 2
                    psum_out = []
                    for nh in range(2):
                        dgh = dg_sb[:, nh * 512:(nh + 1) * 512].rearrange(
                            "p (b f) -> p b f", b=HB)
                        s_red = wpool.tile([128, HB, 1], F32, tag=f"s_red{nh}",
                                           name=f"s_red{nh}")
                        nc.vector.reduce_sum(s_red[:], dgh,
                                             axis=mybir.AxisListType.X)
                        tk_red = []
                        for k in range(2):
                            tmpk = spool.tile([128, HB, 64], DT16, tag="tmpk")
                            cc = ccls[:, k * 64:(k + 1) * 64].unsqueeze(1)
                            nc.vector.tensor_tensor(
                                tmpk[:], dgh, cc.broadcast_to([128, HB, 64]), MUL)
                            tkr = wpool.tile([128, HB, 1], F32,
                                             tag=f"tkr_{nh}_{k}",
                                             name=f"tkr_{nh}_{k}")
                            nc.vector.reduce_sum(tkr[:], tmpk[:],
                                                 axis=mybir.AxisListType.X)
                            tk_red.append(tkr)
                        po = ps2.tile([HB, 5], F32,
                                      tag=("psA" if nh == 0 else "psB"),
                                      name=f"psum_out{nh}")
                        for k in range(2):
                            nc.tensor.matmul(po[0:HB, 2 * k:2 * k + 2],
                                             lhsT=tk_red[k][:], rhs=sel[:],
                                             start=True, stop=True)
                        nc.tensor.matmul(po[0:HB, 4:5], lhsT=s_red[:],
                                         rhs=fusw[:], start=True, stop=True)
                        psum_out.append(po)

                if tail_on:
                    for nh in range(2):
                        osb = wpool.tile([8, 5], F32, tag=f"osb{nh}",
                                         name=f"osb{nh}")
                        nc.vector.tensor_copy(osb[:], psum_out[nh][:])
                        nc.sync.dma_start(out_d[nh * 8:(nh + 1) * 8, :],
                                          osb[:])
                else:
                    out_sb = wpool.tile([16, 5], F32, tag="out_sb")
                    nc.vector.memset(out_sb[:], 0.0)
                    nc.sync.dma_start(out_d[:], out_sb[:])

            if timing_loop is None:
                emit()
            else:
                krep = int(_os.environ.get("KREP", "1"))
                kw = {}
                if _os.environ.get("KSTAG", "0") == "1":
                    kw["staggered_reset"] = True
                if _os.environ.get("KHINT", "0") == "1":
                    kw["hint_engines"] = (mybir.EngineType.PE,
                                          mybir.EngineType.SP,
                                          mybir.EngineType.DVE,
                                          mybir.EngineType.Activation)
                with tc.For_i(0, timing_loop, 1, **kw):
                    for _ in range(krep):
                        emit()

    nc.compile()
    return nc


_NC = {}


def _get_nc(timing_loop=None):
    if timing_loop not in _NC:
        _NC[timing_loop] = _build_nc(timing_loop)
    return _NC[timing_loop]


def _make_consts(conv_dct_w, conv_dct_b, bn_dct_g, bn_dct_b,
                 conv_grad_w, conv_grad_b, bn_grad_g, bn_grad_b,
                 fus_w, cls_w):
    A = build_A()
    consts = {}
    consts["at"] = np.ascontiguousarray(A.T).astype(NP_DT16)
    consts["ident"] = np.eye(8, dtype=np.float32).astype(NP_DT16)

    BN_EPS = 1e-5
    convw = np.zeros((3, 1152), np.float32)
    bias = np.zeros((128, 1), np.float32)
    for br, (w, b, g, beta) in enumerate((
            (conv_dct_w, conv_dct_b, bn_dct_g, bn_dct_b),
            (conv_grad_w, conv_grad_b, bn_grad_g, bn_grad_b))):
        g_eff = (g / np.sqrt(1.0 + BN_EPS)).astype(np.float32)
        w_eff = w * g_eff[:, None, None, None]
        if br == 1:
            w_eff = w_eff * 0.25  # fold the 4-sample average
        b_eff = b * g_eff + beta
        for di in range(3):
            for dj in range(3):
                # convw[ic, (br*9+di*3+dj)*64 + oc] = w_eff[oc, ic, di, dj]
                off = (br * 9 + di * 3 + dj) * 64
                convw[:, off:off + 64] = w_eff[:, :, di, dj].T
        bias[br * 64:(br + 1) * 64, 0] = b_eff
    consts["convw"] = convw.astype(NP_DT16)
    consts["bias"] = bias

    ccls = np.zeros((128, 128), np.float32)
    for k in range(2):
        ccls[0:64, k * 64:(k + 1) * 64] = cls_w[k].reshape(64, 64)
        ccls[64:128, k * 64:(k + 1) * 64] = cls_w[k].reshape(64, 64)
    consts["ccls"] = ccls.astype(NP_DT16)
    consts["fusw"] = np.ascontiguousarray(np.tile(fus_w[0][:, None] / 64.0, (2, 1)))
    sel = np.zeros((128, 2), np.float32)
    sel[0:64, 0] = 1.0
    sel[64:128, 1] = 1.0
    consts["sel"] = sel
    return consts


def kernel_with_results(x, conv_dct_w, conv_dct_b, bn_dct_g, bn_dct_b,
                        conv_grad_w, conv_grad_b, bn_grad_g, bn_grad_b,
                        fus_w, fus_b, cls_w, cls_b, trace=False):
    nc = _get_nc()
    consts = _make_consts(conv_dct_w, conv_dct_b, bn_dct_g, bn_dct_b,
                          conv_grad_w, conv_grad_b, bn_grad_g, bn_grad_b,
                          fus_w, cls_w)
    x = np.ascontiguousarray(np.asarray(x, np.float32))
    in_maps = []
    for i in range(N_CORES):
        m = {"x": np.ascontiguousarray(x[i * B:(i + 1) * B])}
        m.update(consts)
        in_maps.append(m)
    res = run_bass_kernel_spmd(nc, in_maps, list(range(N_CORES)), trace=trace)

    outs = []
    for i in range(N_CORES):
        r = res.results[i]["out"]  # [16, 5]
        Pd = r[:, [0, 2]]
        Pg = r[:, [1, 3]]
        sl = r[:, 4] + np.float32(fus_b[0])
        w = 1.0 / (1.0 + np.exp(-sl))[:, None]
        outs.append(w * Pd + (1.0 - w) * Pg + np.asarray(cls_b)[None, :])
    return np.concatenate(outs, axis=0).astype(np.float32), res


def kernel(**inputs):
    out, _ = kernel_with_results(**inputs)
    return out


# revision 7
# speedup vs baseline: 1.6335x; 1.0719x over previous
"""Trainium2 Bass kernel for nn_AdaptiveFusionNet (8-core data parallel).

Math collapse (validated vs reference):
  - DCT branch: apply_dct(x)[b,c] == A @ X @ A.T with A = R @ D[:,:25] @ D[:25,:]
    (R = 256->8 bilinear resize matrix; note reference's "inverse" einsum
    applies D again, not D.T).
  - Gradient branch: bilinear 256->8 resize samples grad magnitude only at
    rows/cols {15,16}+32g with weight 1/4 each; sobel there needs only the
    4x4 neighborhoods {14..17}+32g (wrap never triggered). So only 32x32
    samples per channel are needed.
  - conv+BN+ReLU folded: w_eff = w*gamma/sqrt(1+eps), b_eff = b*g+beta.
  - Final: out = w*Pd + (1-w)*Pg + cls_b computed on host from per-core
    [16,5] device outputs (Pd0,Pg0,Pd1,Pg1,slogit).

v3 (from v1 trace analysis: fp32 MMs lower to 2 HW passes with ~205ns
128-col weight loads; stage1 was ~82us of PE):
  - whole DCT/conv pipeline in fp16 (FWL weight loads ~52ns, 1-pass MMs,
    DVE 2x); x DMAed f32 then cast per image alternating DVE/ACT.
  - x DMA uses 2-rows-per-partition layout (2KB descriptor runs, half
    the descriptors of the 1KB row-interleave layout); A is split by row
    parity to match.
  - stage2 operand swap: lhsT=w1t_sb rhs=A-half puts (b,i) on psum
    partitions directly, eliminating the PE transposes of v1.
  - PSUM: stage1's six [128,128] groups packed into 2 banks per loop
    parity (4+2; psum2T and the out matmuls reuse spare columns), conv
    psc uses 4 shared banks -> 8 banks total, so consecutive unrolled
    emits (KREP=2, parity-alternating) can overlap.
  - dconv/gconv are persistent pre-zeroed tiles (borders memset once).
  - sobel computed in fp16 (gpatch cast on the ACT column-sample copy).
  - DMA issue spread: xt on SyncE, gp/out on ScalarE, dflat/gflat on
    GpSimd (SWDGE).
"""
import sys

import numpy as np

try:
    import concourse  # noqa: F401
except ImportError:
    sys.path.insert(0, "/opt/trn_rl_repo")

import concourse.bass as bass
import concourse.bacc as bacc
import concourse.mybir as mybir
from concourse import tile
from concourse.bass_utils import run_bass_kernel_spmd

F32 = mybir.dt.float32
import os as _os_dt
DT16 = (mybir.dt.float16 if _os_dt.environ.get("KDT16", "fp16") == "fp16"
        else mybir.dt.bfloat16)
NP_DT16 = mybir.dt.np(DT16)
N_CORES = 8
B_TOTAL = 128
B = B_TOTAL // N_CORES  # 16 images per core
NCH = B * 3             # 48 channels per core


def build_A():
    N = 256
    n = np.arange(N, dtype=np.float64)
    k = n[:, None]
    D = np.cos(np.pi * (2.0 * n + 1.0) * k / (2.0 * N))
    scale = np.full((N, 1), np.sqrt(2.0 / N))
    scale[0, 0] = np.sqrt(1.0 / N)
    D = D * scale
    R = np.zeros((8, 256))
    for i in range(8):
        R[i, 15 + 32 * i] = 0.5
        R[i, 16 + 32 * i] = 0.5
    A = R @ D[:, :25] @ D[:25, :]
    return A.astype(np.float32)


def _build_nc(timing_loop=None):
    import os as _os
    drop = set(_os.environ.get("KDROP", "").split(","))
    nc = bacc.Bacc("TRN2", target_bir_lowering=False, debug=False,
                   num_devices=N_CORES)

    if timing_loop is None:
        x_d = nc.dram_tensor("x", [B, 3, 256, 256], F32, kind="ExternalInput")
    else:
        x_d = nc.dram_tensor("xint", [B, 3, 256, 256], F32)
    # at: rows 0:128 = A.T[0::2] (even src rows), 128:256 = A.T[1::2],
    #     256:384 = A.T[0:128], 384:512 = A.T[128:256] (stage2 halves)
    at_d = nc.dram_tensor("at", [512, 8], DT16, kind="ExternalInput")
    convw_d = nc.dram_tensor("convw", [3, 1152], DT16, kind="ExternalInput")
    bias_d = nc.dram_tensor("bias", [128, 1], F32, kind="ExternalInput")
    ccls_d = nc.dram_tensor("ccls", [128, 128], DT16, kind="ExternalInput")
    fusw_d = nc.dram_tensor("fusw", [128, 1], F32, kind="ExternalInput")
    sel_d = nc.dram_tensor("sel", [128, 2], F32, kind="ExternalInput")
    out_d = nc.dram_tensor("out", [B, 5], F32, kind="ExternalOutput")

    with tile.TileContext(nc) as tc:
        with (
            tc.tile_pool(name="const", bufs=1) as cpool,
            tc.tile_pool(name="xin", bufs=6) as xpool,
            tc.tile_pool(name="xbf", bufs=4) as xbpool,
            tc.tile_pool(name="work", bufs=1) as wpool,
            tc.tile_pool(name="scratch", bufs=2) as spool,
            tc.tile_pool(name="ps1", bufs=1, space="PSUM") as ps1,
            tc.tile_pool(name="ps2", bufs=1, space="PSUM") as ps2,
        ):
            # ---- constants ----
            a1e = cpool.tile([128, 8], DT16, tag="a1e")
            a1o = cpool.tile([128, 8], DT16, tag="a1o")
            a1c = cpool.tile([128, 8], DT16, tag="a1c")
            a2c = cpool.tile([128, 8], DT16, tag="a2c")
            nc.sync.dma_start(a1e[:], at_d[0:128, :])
            nc.sync.dma_start(a1o[:], at_d[128:256, :])
            nc.sync.dma_start(a1c[:], at_d[256:384, :])
            nc.sync.dma_start(a2c[:], at_d[384:512, :])
            convw = cpool.tile([3, 1152], DT16, tag="convw")
            nc.sync.dma_start(convw[:], convw_d[:])
            biasd = cpool.tile([64, 1], F32, tag="biasd")
            biasg = cpool.tile([64, 1], F32, tag="biasg")
            nc.sync.dma_start(biasd[:], bias_d[0:64, :])
            nc.sync.dma_start(biasg[:], bias_d[64:128, :])
            ccls = cpool.tile([128, 128], DT16, tag="ccls")
            nc.sync.dma_start(ccls[:], ccls_d[:])
            fusw = cpool.tile([128, 1], F32, tag="fusw")
            nc.sync.dma_start(fusw[:], fusw_d[:])
            sel = cpool.tile([128, 2], F32, tag="sel")
            nc.sync.dma_start(sel[:], sel_d[:])
            # persistent zero-bordered conv inputs, one pair per parity
            convin = [[cpool.tile([3, B, 10, 10], DT16, tag=f"cvin{p}_{br}",
                                  name=f"cvin{p}_{br}")
                       for br in range(2)] for p in range(2)]
            for p in range(2):
                for br in range(2):
                    nc.vector.memset(convin[p][br][:], 0.0)

            SUB = mybir.AluOpType.subtract
            ADD = mybir.AluOpType.add
            MUL = mybir.AluOpType.mult

            def emit(par):
                # ---- PSUM banks for this parity ----
                bankA = ps1.tile([128, 512], F32, tag=f"psA{par}",
                                 name=f"bankA{par}")
                bankB = ps1.tile([128, 512], F32, tag=f"psB{par}",
                                 name=f"bankB{par}")

                def w1t_view(ic, cb):
                    if ic < 2:
                        off = (ic * 2 + cb) * 128
                        return bankA[:, off:off + 128]
                    return bankB[:, cb * 128:cb * 128 + 128]

                # ---- stage 1: X^T A^T per channel (fp16) ----
                for b in range(B):
                    xt = xpool.tile([128, 3, 2, 256], F32, tag="xt")
                    nc.sync.dma_start(
                        xt[:], x_d[b].rearrange("c (p r2) w -> p c r2 w", r2=2))
                    xtb = xbpool.tile([128, 3, 2, 256], DT16, tag="xtb")
                    if b % 3 == 2:
                        nc.scalar.copy(xtb[:], xt[:])
                    else:
                        nc.vector.tensor_copy(xtb[:], xt[:])
                    for ic in range(3):
                        for cb in range(2):
                            for r2 in range(2):
                                nc.tensor.matmul(
                                    w1t_view(ic, cb)[:, b * 8:(b + 1) * 8],
                                    lhsT=xtb[:, ic, r2, cb * 128:(cb + 1) * 128],
                                    rhs=(a1e[:] if r2 == 0 else a1o[:]),
                                    start=(r2 == 0), stop=(r2 == 1))

                # ---- gradient branch: row gather DMAs (ic-major partitions) ----
                gp = wpool.tile([NCH, 8, 4, 256], F32, tag=f"gp{par}")
                if "gpdma" not in drop:
                    for ic in range(3):
                        src = x_d[:, ic].rearrange(
                            "b (g h) w -> b g h w", h=32)[:, :, 14:18, :]
                        nc.scalar.dma_start(gp[ic * B:(ic + 1) * B], src)

                # ---- stage 2 (swapped): psum2T[(b,i), ic*8+j] ----
                dcttail = "dcttail" not in drop
                w1t_sb = [[wpool.tile([128, 128], DT16,
                                      tag=f"w1sb{par}_{ic}_{cb}",
                                      name=f"w1sb{par}_{ic}_{cb}")
                           for cb in range(2)] for ic in range(3)]
                if dcttail:
                    for ic in range(3):
                        for cb in range(2):
                            nc.vector.tensor_copy(w1t_sb[ic][cb][:],
                                                  w1t_view(ic, cb))
                psum2T = bankB[:, 256:280]  # [128, 24] in bankB spare cols
                dt_sb = wpool.tile([128, 24], DT16, tag=f"dt_sb{par}")
                dflat = wpool.tile([3, 1024], DT16, tag=f"dflat{par}")
                dconv = convin[par][0]
                if dcttail:
                    for ic in range(3):
                        for cb in range(2):
                            nc.tensor.matmul(
                                psum2T[:, ic * 8:(ic + 1) * 8],
                                lhsT=w1t_sb[ic][cb][:],
                                rhs=(a1c[:] if cb == 0 else a2c[:]),
                                start=(cb == 0), stop=(cb == 1))
                    nc.vector.tensor_copy(dt_sb[:], psum2T)
                    for ic in range(3):
                        nc.gpsimd.dma_start(dflat[ic:ic + 1, :],
                                            dt_sb[:, ic * 8:(ic + 1) * 8])
                    nc.vector.tensor_copy(
                        dconv[:, :, 1:9, 1:9],
                        dflat.rearrange("c (b i j) -> c b i j", b=B, i=8))

                # ---- gradient branch: col sample + sobel (fp16) ----
                gpatch = wpool.tile([NCH, 8, 4, 8, 4], DT16, tag=f"gpatch{par}")
                g8 = wpool.tile([NCH, 8, 8, 1], DT16, tag=f"g8{par}")
                gflat = wpool.tile([3, 1024], DT16, tag=f"gflat{par}")
                gconv = convin[par][1]
                if "sobel" not in drop:
                    gp5 = gp.rearrange("p g r (gc c) -> p g r gc c", c=32)
                    for g in range(8):
                        nc.scalar.copy(gpatch[:, g], gp5[:, g, :, :, 14:18])

                    def PP(r, dc):
                        return gpatch[:, :, r, :, 1 + dc:3 + dc]  # [48, 8, 8, 2]

                    a_lr = []
                    for lr in (1, 2):
                        t1 = spool.tile([NCH, 8, 8, 2], DT16, tag="t1")
                        t2 = spool.tile([NCH, 8, 8, 2], DT16, tag="t2")
                        t3 = spool.tile([NCH, 8, 8, 2], DT16, tag="t3")
                        nc.vector.tensor_tensor(t1[:], PP(lr - 1, 1), PP(lr - 1, -1), SUB)
                        nc.vector.tensor_tensor(t2[:], PP(lr, 1), PP(lr, -1), SUB)
                        nc.vector.tensor_tensor(t3[:], PP(lr + 1, 1), PP(lr + 1, -1), SUB)
                        u = spool.tile([NCH, 8, 8, 2], DT16, tag="u")
                        nc.vector.tensor_tensor(u[:], t1[:], t3[:], ADD)
                        gx = spool.tile([NCH, 8, 8, 2], DT16, tag="gx")
                        nc.vector.scalar_tensor_tensor(gx[:], t2[:], 2.0, u[:], MUL, ADD)
                        s1 = spool.tile([NCH, 8, 8, 2], DT16, tag="s1")
                        s2 = spool.tile([NCH, 8, 8, 2], DT16, tag="s2")
                        s3 = spool.tile([NCH, 8, 8, 2], DT16, tag="s3")
                        nc.vector.tensor_tensor(s1[:], PP(lr + 1, -1), PP(lr - 1, -1), SUB)
                        nc.vector.tensor_tensor(s2[:], PP(lr + 1, 0), PP(lr - 1, 0), SUB)
                        nc.vector.tensor_tensor(s3[:], PP(lr + 1, 1), PP(lr - 1, 1), SUB)
                        u2 = spool.tile([NCH, 8, 8, 2], DT16, tag="u2")
                        nc.vector.tensor_tensor(u2[:], s1[:], s3[:], ADD)
                        gy = spool.tile([NCH, 8, 8, 2], DT16, tag="gy")
                        nc.vector.scalar_tensor_tensor(gy[:], s2[:], 2.0, u2[:], MUL, ADD)
                        nc.vector.tensor_tensor(gx[:], gx[:], gx[:], MUL)
                        nc.vector.tensor_tensor(gy[:], gy[:], gy[:], MUL)
                        m2 = spool.tile([NCH, 8, 8, 2], DT16, tag="m2")
                        nc.vector.tensor_tensor(m2[:], gx[:], gy[:], ADD)
                        mag = spool.tile([NCH, 8, 8, 2], DT16, tag=f"mag{lr}")
                        nc.scalar.sqrt(mag[:], m2[:])
                        al = spool.tile([NCH, 8, 8, 1], DT16, tag=f"al{lr}")
                        nc.vector.tensor_tensor(al[:], mag[:, :, :, 0:1], mag[:, :, :, 1:2], ADD)
                        a_lr.append(al)

                    nc.vector.tensor_tensor(g8[:], a_lr[0][:], a_lr[1][:], ADD)
                    g8f = g8.rearrange("p a b c -> p (a b c)")  # [48, 64]

                    for ic in range(3):
                        nc.gpsimd.dma_start(gflat[ic:ic + 1, :],
                                            g8f[ic * B:(ic + 1) * B, :])
                    nc.vector.tensor_copy(
                        gconv[:, :, 1:9, 1:9],
                        gflat.rearrange("c (b i j) -> c b i j", b=B, i=8))

                # ---- convs (fp16, shared psum banks) ----
                psc = [[ps1.tile([64, 512], F32, tag=f"psc_{br}_{nh}",
                                 name=f"psc{par}_{br}_{nh}") for nh in range(2)]
                       for br in range(2)]
                if not ("conv" in drop or "tail" in drop):
                    for nh in range(2):
                        for br, rhs_t in ((0, dconv), (1, gconv)):
                            for si, (di, dj) in enumerate(
                                    (di, dj) for di in range(3) for dj in range(3)):
                                w_off = (br * 9 + di * 3 + dj) * 64
                                rv = rhs_t[:, nh * 8:(nh + 1) * 8,
                                           di:di + 8, dj:dj + 8]
                                nc.tensor.matmul(
                                    psc[br][nh][:, :],
                                    lhsT=convw[:, w_off:w_off + 64],
                                    rhs=rv,
                                    start=(si == 0), stop=(si == 8))

                # relu(conv + bias) -> dg_sb [128, 1024] fp16
                tail_on = "tail" not in drop
                dg_sb = wpool.tile([128, 1024], DT16, tag=f"dg_sb{par}")
                if tail_on:
                    for nh in range(2):
                        for br in range(2):
                            nc.scalar.activation(
                                dg_sb[br * 64:(br + 1) * 64,
                                      nh * 512:(nh + 1) * 512],
                                psc[br][nh][:],
                                mybir.ActivationFunctionType.Relu,
                                bias=(biasd[:] if br == 0 else biasg[:]),
                                scale=1.0)

                    # ---- fusion + classifier (per half for pipelining) ----
                    HB = B // 2
                    psum_out = []
                    for nh in range(2):
                        dgh = dg_sb[:, nh * 512:(nh + 1) * 512].rearrange(
                            "p (b f) -> p b f", b=HB)
                        s_red = wpool.tile([128, HB, 1], F32,
                                           tag=f"s_red{par}{nh}",
                                           name=f"s_red{par}{nh}")
                        nc.vector.reduce_sum(s_red[:], dgh,
                                             axis=mybir.AxisListType.X)
                        tk_red = []
                        for k in range(2):
                            tmpk = spool.tile([128, HB, 64], DT16, tag="tmpk")
                            cc = ccls[:, k * 64:(k + 1) * 64].unsqueeze(1)
                            nc.vector.tensor_tensor(
                                tmpk[:], dgh, cc.broadcast_to([128, HB, 64]), MUL)
                            tkr = wpool.tile([128, HB, 1], F32,
                                             tag=f"tkr{par}_{nh}_{k}",
                                             name=f"tkr{par}_{nh}_{k}")
                            nc.vector.reduce_sum(tkr[:], tmpk[:],
                                                 axis=mybir.AxisListType.X)
                            tk_red.append(tkr)
                        po = bankB[0:HB, 288 + 16 * nh:293 + 16 * nh]
                        for k in range(2):
                            nc.tensor.matmul(po[0:HB, 2 * k:2 * k + 2],
                                             lhsT=tk_red[k][:], rhs=sel[:],
                                             start=True, stop=True)
                        nc.tensor.matmul(po[0:HB, 4:5], lhsT=s_red[:],
                                         rhs=fusw[:], start=True, stop=True)
                        psum_out.append(po)

                if tail_on:
                    for nh in range(2):
                        osb = wpool.tile([8, 5], F32, tag=f"osb{par}{nh}",
                                         name=f"osb{par}{nh}")
                        nc.vector.tensor_copy(osb[:], psum_out[nh])
                        nc.scalar.dma_start(out_d[nh * 8:(nh + 1) * 8, :],
                                            osb[:])
                else:
                    out_sb = wpool.tile([16, 5], F32, tag=f"out_sb{par}")
                    nc.vector.memset(out_sb[:], 0.0)
                    nc.scalar.dma_start(out_d[:], out_sb[:])

            if timing_loop is None:
                emit(0)
            else:
                krep = int(_os.environ.get("KREP", "2"))
                kw = {}
                if _os.environ.get("KSTAG", "0") == "1":
                    kw["staggered_reset"] = True
                if _os.environ.get("KHINT", "0") == "1":
                    kw["hint_engines"] = (mybir.EngineType.PE,
                                          mybir.EngineType.SP,
                                          mybir.EngineType.DVE,
                                          mybir.EngineType.Activation)
                with tc.For_i(0, timing_loop, 1, **kw):
                    for r in range(krep):
                        emit(r % 2)

    nc.compile()
    return nc


_NC = {}


def _get_nc(timing_loop=None):
    if timing_loop not in _NC:
        _NC[timing_loop] = _build_nc(timing_loop)
    return _NC[timing_loop]


def _make_consts(conv_dct_w, conv_dct_b, bn_dct_g, bn_dct_b,
                 conv_grad_w, conv_grad_b, bn_grad_g, bn_grad_b,
                 fus_w, cls_w):
    A = build_A()
    AT = np.ascontiguousarray(A.T)  # [256, 8]
    consts = {}
    at = np.zeros((512, 8), np.float32)
    at[0:128] = AT[0::2]
    at[128:256] = AT[1::2]
    at[256:384] = AT[0:128]
    at[384:512] = AT[128:256]
    consts["at"] = at.astype(NP_DT16)

    BN_EPS = 1e-5
    convw = np.zeros((3, 1152), np.float32)
    bias = np.zeros((128, 1), np.float32)
    for br, (w, b, g, beta) in enumerate((
            (conv_dct_w, conv_dct_b, bn_dct_g, bn_dct_b),
            (conv_grad_w, conv_grad_b, bn_grad_g, bn_grad_b))):
        g_eff = (g / np.sqrt(1.0 + BN_EPS)).astype(np.float32)
        w_eff = w * g_eff[:, None, None, None]
        if br == 1:
            w_eff = w_eff * 0.25  # fold the 4-sample average
        b_eff = b * g_eff + beta
        for di in range(3):
            for dj in range(3):
                # convw[ic, (br*9+di*3+dj)*64 + oc] = w_eff[oc, ic, di, dj]
                off = (br * 9 + di * 3 + dj) * 64
                convw[:, off:off + 64] = w_eff[:, :, di, dj].T
        bias[br * 64:(br + 1) * 64, 0] = b_eff
    consts["convw"] = convw.astype(NP_DT16)
    consts["bias"] = bias

    ccls = np.zeros((128, 128), np.float32)
    for k in range(2):
        ccls[0:64, k * 64:(k + 1) * 64] = cls_w[k].reshape(64, 64)
        ccls[64:128, k * 64:(k + 1) * 64] = cls_w[k].reshape(64, 64)
    consts["ccls"] = ccls.astype(NP_DT16)
    consts["fusw"] = np.ascontiguousarray(np.tile(fus_w[0][:, None] / 64.0, (2, 1)))
    sel = np.zeros((128, 2), np.float32)
    sel[0:64, 0] = 1.0
    sel[64:128, 1] = 1.0
    consts["sel"] = sel
    return consts


def kernel_with_results(x, conv_dct_w, conv_dct_b, bn_dct_g, bn_dct_b,
                        conv_grad_w, conv_grad_b, bn_grad_g, bn_grad_b,
                        fus_w, fus_b, cls_w, cls_b, trace=False):
    nc = _get_nc()
    consts = _make_consts(conv_dct_w, conv_dct_b, bn_dct_g, bn_dct_b,
                          conv_grad_w, conv_grad_b, bn_grad_g, bn_grad_b,
                          fus_w, cls_w)
    x = np.ascontiguousarray(np.asarray(x, np.float32))
    in_maps = []
    for i in range(N_CORES):
        m = {"x": np.ascontiguousarray(x[i * B:(i + 1) * B])}
        m.update(consts)
        in_maps.append(m)
    res = run_bass_kernel_spmd(nc, in_maps, list(range(N_CORES)), trace=trace)

    outs = []
    for i in range(N_CORES):
        r = res.results[i]["out"]  # [16, 5]
        Pd = r[:, [0, 2]]
        Pg = r[:, [1, 3]]
        sl = r[:, 4] + np.float32(fus_b[0])
        w = 1.0 / (1.0 + np.exp(-sl))[:, None]
        outs.append(w * Pd + (1.0 - w) * Pg + np.asarray(cls_b)[None, :])
    return np.concatenate(outs, axis=0).astype(np.float32), res


def kernel(**inputs):
    out, _ = kernel_with_results(**inputs)
    return out


# revision 14
# speedup vs baseline: 1.9321x; 1.1828x over previous
"""Trainium2 Bass kernel for nn_AdaptiveFusionNet (8-core data parallel).

Math collapse (validated vs reference):
  - DCT branch: apply_dct(x)[b,c] == A @ X @ A.T with A = R @ D[:,:25] @ D[:25,:]
    (R = 256->8 bilinear resize matrix; note reference's "inverse" einsum
    applies D again, not D.T).
  - Gradient branch: bilinear 256->8 resize samples grad magnitude only at
    rows/cols {15,16}+32g with weight 1/4 each; sobel there needs only the
    4x4 neighborhoods {14..17}+32g (wrap never triggered). So only 32x32
    samples per channel are needed.
  - conv+BN+ReLU folded: w_eff = w*gamma/sqrt(1+eps), b_eff = b*g+beta.
  - Final: out = w*Pd + (1-w)*Pg + cls_b computed on host from per-core
    [16,5] device outputs (Pd0,Pg0,Pd1,Pg1,slogit).

v3 (from v1 trace analysis: fp32 MMs lower to 2 HW passes with ~205ns
128-col weight loads; stage1 was ~82us of PE):
  - whole DCT/conv pipeline in fp16 (FWL weight loads ~52ns, 1-pass MMs,
    DVE 2x); x DMAed f32 then cast per image alternating DVE/ACT.
  - x DMA uses 2-rows-per-partition layout (2KB descriptor runs, half
    the descriptors of the 1KB row-interleave layout); A is split by row
    parity to match.
  - stage2 operand swap: lhsT=w1t_sb rhs=A-half puts (b,i) on psum
    partitions directly, eliminating the PE transposes of v1.
  - PSUM: stage1's six [128,128] groups packed into 2 banks per loop
    parity (4+2; psum2T and the out matmuls reuse spare columns), conv
    psc uses 4 shared banks -> 8 banks total, so consecutive unrolled
    emits (KREP=2, parity-alternating) can overlap.
  - dconv/gconv are persistent pre-zeroed tiles (borders memset once).
  - sobel computed in fp16 (gpatch cast on the ACT column-sample copy).
  - DMA issue spread: xt on SyncE, gp/out on ScalarE, dflat/gflat on
    GpSimd (SWDGE).
"""
import sys

import numpy as np

try:
    import concourse  # noqa: F401
except ImportError:
    sys.path.insert(0, "/opt/trn_rl_repo")

import concourse.bass as bass
import concourse.bacc as bacc
import concourse.mybir as mybir
from concourse import tile
from concourse.bass_utils import run_bass_kernel_spmd

F32 = mybir.dt.float32
import os as _os_dt
DT16 = (mybir.dt.float16 if _os_dt.environ.get("KDT16", "fp16") == "fp16"
        else mybir.dt.bfloat16)
NP_DT16 = mybir.dt.np(DT16)
N_CORES = 8
B_TOTAL = 128
B = B_TOTAL // N_CORES  # 16 images per core
NCH = B * 3             # 48 channels per core


def build_A():
    N = 256
    n = np.arange(N, dtype=np.float64)
    k = n[:, None]
    D = np.cos(np.pi * (2.0 * n + 1.0) * k / (2.0 * N))
    scale = np.full((N, 1), np.sqrt(2.0 / N))
    scale[0, 0] = np.sqrt(1.0 / N)
    D = D * scale
    R = np.zeros((8, 256))
    for i in range(8):
        R[i, 15 + 32 * i] = 0.5
        R[i, 16 + 32 * i] = 0.5
    A = R @ D[:, :25] @ D[:25, :]
    return A.astype(np.float32)


def _build_nc(timing_loop=None):
    import os as _os
    drop = set(_os.environ.get("KDROP", "").split(","))
    nc = bacc.Bacc("TRN2", target_bir_lowering=False, debug=False,
                   num_devices=N_CORES)

    if timing_loop is None:
        x_d = nc.dram_tensor("x", [B, 3, 256, 256], F32, kind="ExternalInput")
    else:
        x_d = nc.dram_tensor("xint", [B, 3, 256, 256], F32)
    # at: rows 0:128 = A.T[0::2] (even src rows), 128:256 = A.T[1::2],
    #     256:384 = A.T[0:128], 384:512 = A.T[128:256] (stage2 halves)
    at_d = nc.dram_tensor("at", [512, 8], DT16, kind="ExternalInput")
    convw_d = nc.dram_tensor("convw", [3, 1152], DT16, kind="ExternalInput")
    bias_d = nc.dram_tensor("bias", [128, 1], F32, kind="ExternalInput")
    ccls_d = nc.dram_tensor("ccls", [128, 128], DT16, kind="ExternalInput")
    fusw_d = nc.dram_tensor("fusw", [128, 1], F32, kind="ExternalInput")
    sel_d = nc.dram_tensor("sel", [128, 2], F32, kind="ExternalInput")
    out_d = nc.dram_tensor("out", [B, 5], F32, kind="ExternalOutput")

    with tile.TileContext(nc) as tc:
        with (
            tc.tile_pool(name="const", bufs=1) as cpool,
            tc.tile_pool(name="xin", bufs=6) as xpool,
            tc.tile_pool(name="xbf", bufs=8) as xbpool,
            tc.tile_pool(name="work", bufs=1) as wpool,
            tc.tile_pool(name="scratch", bufs=2) as spool,
            tc.tile_pool(name="ps1", bufs=1, space="PSUM") as ps1,
            tc.tile_pool(name="ps2", bufs=1, space="PSUM") as ps2,
        ):
            # ---- constants ----
            a1e = cpool.tile([128, 8], DT16, tag="a1e")
            a1o = cpool.tile([128, 8], DT16, tag="a1o")
            a1c = cpool.tile([128, 8], DT16, tag="a1c")
            a2c = cpool.tile([128, 8], DT16, tag="a2c")
            nc.sync.dma_start(a1e[:], at_d[0:128, :])
            nc.sync.dma_start(a1o[:], at_d[128:256, :])
            nc.sync.dma_start(a1c[:], at_d[256:384, :])
            nc.sync.dma_start(a2c[:], at_d[384:512, :])
            convw = cpool.tile([3, 1152], DT16, tag="convw")
            nc.sync.dma_start(convw[:], convw_d[:])
            biasd = cpool.tile([64, 1], F32, tag="biasd")
            biasg = cpool.tile([64, 1], F32, tag="biasg")
            nc.sync.dma_start(biasd[:], bias_d[0:64, :])
            nc.sync.dma_start(biasg[:], bias_d[64:128, :])
            ccls = cpool.tile([128, 128], DT16, tag="ccls")
            nc.sync.dma_start(ccls[:], ccls_d[:])
            fusw = cpool.tile([128, 1], F32, tag="fusw")
            nc.sync.dma_start(fusw[:], fusw_d[:])
            sel = cpool.tile([128, 2], F32, tag="sel")
            nc.sync.dma_start(sel[:], sel_d[:])
            # persistent zero-bordered conv inputs, one pair per parity
            convin = [[cpool.tile([3, B, 10, 10], DT16, tag=f"cvin{p}_{br}",
                                  name=f"cvin{p}_{br}")
                       for br in range(2)] for p in range(2)]
            for p in range(2):
                for br in range(2):
                    nc.vector.memset(convin[p][br][:], 0.0)

            SUB = mybir.AluOpType.subtract
            ADD = mybir.AluOpType.add
            MUL = mybir.AluOpType.mult

            kxsw = _os.environ.get("KXSW", "1") == "1"

            def emit(par):
                # ---- PSUM banks for this parity ----
                bankA = ps1.tile([128, 512], F32, tag=f"psA{par}",
                                 name=f"bankA{par}")
                bankB = ps1.tile([128, 512], F32, tag=f"psB{par}",
                                 name=f"bankB{par}")

                def w1t_view(ic, cb):
                    if ic < 2:
                        off = (ic * 2 + cb) * 128
                        return bankA[:, off:off + 128]
                    return bankB[:, cb * 128:cb * 128 + 128]

                # ---- stage 1: X^T A^T per channel (fp16) ----
                for b in range(B):
                    xtb = xbpool.tile([128, 3, 2, 256], DT16, tag="xtb")
                    xv = x_d[b].rearrange("c (p r2) w -> p c r2 w", r2=2)
                    if kxsw:
                        nc.gpsimd.dma_start(xtb[:], xv)  # SWDGE casts f32->f16
                    else:
                        xt = xpool.tile([128, 3, 2, 256], F32, tag="xt")
                        nc.sync.dma_start(xt[:], xv)
                        if b % 3 == 2:
                            nc.scalar.copy(xtb[:], xt[:])
                        else:
                            nc.vector.tensor_copy(xtb[:], xt[:])
                    for ic in range(3):
                        for cb in range(2):
                            for r2 in range(2):
                                nc.tensor.matmul(
                                    w1t_view(ic, cb)[:, b * 8:(b + 1) * 8],
                                    lhsT=xtb[:, ic, r2, cb * 128:(cb + 1) * 128],
                                    rhs=(a1e[:] if r2 == 0 else a1o[:]),
                                    start=(r2 == 0), stop=(r2 == 1))

                # ---- gradient branch: row gather DMAs (ic-major partitions) ----
                gp = wpool.tile([NCH, 8, 4, 256], DT16, tag=f"gp{par}")
                if "gpdma" not in drop:
                    for ic in range(3):
                        src = x_d[:, ic].rearrange(
                            "b (g h) w -> b g h w", h=32)[:, :, 14:18, :]
                        nc.gpsimd.dma_start(gp[ic * B:(ic + 1) * B], src)

                # ---- stage 2 (swapped): psum2T[(b,i), ic*8+j] ----
                dcttail = "dcttail" not in drop
                w1t_sb = [[wpool.tile([128, 128], DT16,
                                      tag=f"w1sb{par}_{ic}_{cb}",
                                      name=f"w1sb{par}_{ic}_{cb}")
                           for cb in range(2)] for ic in range(3)]
                if dcttail:
                    for ic in range(3):
                        for cb in range(2):
                            nc.vector.tensor_copy(w1t_sb[ic][cb][:],
                                                  w1t_view(ic, cb))
                psum2T = bankB[:, 256:280]  # [128, 24] in bankB spare cols
                dt_sb = wpool.tile([128, 24], DT16, tag=f"dt_sb{par}")
                dflat = wpool.tile([3, 1024], DT16, tag=f"dflat{par}")
                dconv = convin[par][0]
                if dcttail:
                    for ic in range(3):
                        for cb in range(2):
                            nc.tensor.matmul(
                                psum2T[:, ic * 8:(ic + 1) * 8],
                                lhsT=w1t_sb[ic][cb][:],
                                rhs=(a1c[:] if cb == 0 else a2c[:]),
                                start=(cb == 0), stop=(cb == 1))
                    nc.vector.tensor_copy(dt_sb[:], psum2T)
                    for ic in range(3):
                        nc.sync.dma_start(dflat[ic:ic + 1, :],
                                          dt_sb[:, ic * 8:(ic + 1) * 8])
                    nc.vector.tensor_copy(
                        dconv[:, :, 1:9, 1:9],
                        dflat.rearrange("c (b i j) -> c b i j", b=B, i=8))

                # ---- gradient branch: col sample + sobel (fp16) ----
                gpatch = wpool.tile([NCH, 8, 4, 8, 4], DT16, tag=f"gpatch{par}")
                g8 = wpool.tile([NCH, 8, 8, 1], DT16, tag=f"g8{par}")
                gflat = wpool.tile([3, 1024], DT16, tag=f"gflat{par}")
                gconv = convin[par][1]
                if "sobel" not in drop:
                    gp5 = gp.rearrange("p g r (gc c) -> p g r gc c", c=32)
                    for g in range(8):
                        nc.scalar.copy(gpatch[:, g], gp5[:, g, :, :, 14:18])

                    def PP(r, dc):
                        return gpatch[:, :, r, :, 1 + dc:3 + dc]  # [48, 8, 8, 2]

                    a_lr = []
                    for lr in (1, 2):
                        t1 = spool.tile([NCH, 8, 8, 2], DT16, tag="t1")
                        t2 = spool.tile([NCH, 8, 8, 2], DT16, tag="t2")
                        t3 = spool.tile([NCH, 8, 8, 2], DT16, tag="t3")
                        nc.vector.tensor_tensor(t1[:], PP(lr - 1, 1), PP(lr - 1, -1), SUB)
                        nc.vector.tensor_tensor(t2[:], PP(lr, 1), PP(lr, -1), SUB)
                        nc.vector.tensor_tensor(t3[:], PP(lr + 1, 1), PP(lr + 1, -1), SUB)
                        u = spool.tile([NCH, 8, 8, 2], DT16, tag="u")
                        nc.vector.tensor_tensor(u[:], t1[:], t3[:], ADD)
                        gx = spool.tile([NCH, 8, 8, 2], DT16, tag="gx")
                        nc.vector.scalar_tensor_tensor(gx[:], t2[:], 2.0, u[:], MUL, ADD)
                        s1 = spool.tile([NCH, 8, 8, 2], DT16, tag="s1")
                        s2 = spool.tile([NCH, 8, 8, 2], DT16, tag="s2")
                        s3 = spool.tile([NCH, 8, 8, 2], DT16, tag="s3")
                        nc.vector.tensor_tensor(s1[:], PP(lr + 1, -1), PP(lr - 1, -1), SUB)
                        nc.vector.tensor_tensor(s2[:], PP(lr + 1, 0), PP(lr - 1, 0), SUB)
                        nc.vector.tensor_tensor(s3[:], PP(lr + 1, 1), PP(lr - 1, 1), SUB)
                        u2 = spool.tile([NCH, 8, 8, 2], DT16, tag="u2")
                        nc.vector.tensor_tensor(u2[:], s1[:], s3[:], ADD)
                        gy = spool.tile([NCH, 8, 8, 2], DT16, tag="gy")
                        nc.vector.scalar_tensor_tensor(gy[:], s2[:], 2.0, u2[:], MUL, ADD)
                        nc.vector.tensor_tensor(gx[:], gx[:], gx[:], MUL)
                        nc.vector.tensor_tensor(gy[:], gy[:], gy[:], MUL)
                        m2 = spool.tile([NCH, 8, 8, 2], DT16, tag="m2")
                        nc.vector.tensor_tensor(m2[:], gx[:], gy[:], ADD)
                        mag = spool.tile([NCH, 8, 8, 2], DT16, tag=f"mag{lr}")
                        nc.scalar.sqrt(mag[:], m2[:])
                        al = spool.tile([NCH, 8, 8, 1], DT16, tag=f"al{lr}")
                        nc.vector.tensor_tensor(al[:], mag[:, :, :, 0:1], mag[:, :, :, 1:2], ADD)
                        a_lr.append(al)

                    nc.vector.tensor_tensor(g8[:], a_lr[0][:], a_lr[1][:], ADD)
                    g8f = g8.rearrange("p a b c -> p (a b c)")  # [48, 64]

                    for ic in range(3):
                        nc.sync.dma_start(gflat[ic:ic + 1, :],
                                          g8f[ic * B:(ic + 1) * B, :])
                    nc.vector.tensor_copy(
                        gconv[:, :, 1:9, 1:9],
                        gflat.rearrange("c (b i j) -> c b i j", b=B, i=8))

                # ---- convs (fp16, shared psum banks) ----
                psc = [[ps1.tile([64, 512], F32, tag=f"psc_{br}_{nh}",
                                 name=f"psc{par}_{br}_{nh}") for nh in range(2)]
                       for br in range(2)]
                if not ("conv" in drop or "tail" in drop):
                    for nh in range(2):
                        for br, rhs_t in ((0, dconv), (1, gconv)):
                            for si, (di, dj) in enumerate(
                                    (di, dj) for di in range(3) for dj in range(3)):
                                w_off = (br * 9 + di * 3 + dj) * 64
                                rv = rhs_t[:, nh * 8:(nh + 1) * 8,
                                           di:di + 8, dj:dj + 8]
                                nc.tensor.matmul(
                                    psc[br][nh][:, :],
                                    lhsT=convw[:, w_off:w_off + 64],
                                    rhs=rv,
                                    start=(si == 0), stop=(si == 8))

                # relu(conv + bias) -> dg_sb [128, 1024] fp16
                tail_on = "tail" not in drop
                dg_sb = wpool.tile([128, 1024], DT16, tag=f"dg_sb{par}")
                if tail_on:
                    for nh in range(2):
                        for br in range(2):
                            nc.scalar.activation(
                                dg_sb[br * 64:(br + 1) * 64,
                                      nh * 512:(nh + 1) * 512],
                                psc[br][nh][:],
                                mybir.ActivationFunctionType.Relu,
                                bias=(biasd[:] if br == 0 else biasg[:]),
                                scale=1.0)

                    # ---- fusion + classifier (per half for pipelining) ----
                    HB = B // 2
                    psum_out = []
                    for nh in range(2):
                        dgh = dg_sb[:, nh * 512:(nh + 1) * 512].rearrange(
                            "p (b f) -> p b f", b=HB)
                        s_red = wpool.tile([128, HB, 1], F32,
                                           tag=f"s_red{par}{nh}",
                                           name=f"s_red{par}{nh}")
                        nc.vector.reduce_sum(s_red[:], dgh,
                                             axis=mybir.AxisListType.X)
                        tk_red = []
                        for k in range(2):
                            tmpk = spool.tile([128, HB, 64], DT16, tag="tmpk")
                            cc = ccls[:, k * 64:(k + 1) * 64].unsqueeze(1)
                            nc.vector.tensor_tensor(
                                tmpk[:], dgh, cc.broadcast_to([128, HB, 64]), MUL)
                            tkr = wpool.tile([128, HB, 1], F32,
                                             tag=f"tkr{par}_{nh}_{k}",
                                             name=f"tkr{par}_{nh}_{k}")
                            nc.vector.reduce_sum(tkr[:], tmpk[:],
                                                 axis=mybir.AxisListType.X)
                            tk_red.append(tkr)
                        po = bankB[0:HB, 288 + 16 * nh:293 + 16 * nh]
                        for k in range(2):
                            nc.tensor.matmul(po[0:HB, 2 * k:2 * k + 2],
                                             lhsT=tk_red[k][:], rhs=sel[:],
                                             start=True, stop=True)
                        nc.tensor.matmul(po[0:HB, 4:5], lhsT=s_red[:],
                                         rhs=fusw[:], start=True, stop=True)
                        psum_out.append(po)

                if tail_on:
                    for nh in range(2):
                        osb = wpool.tile([8, 5], F32, tag=f"osb{par}{nh}",
                                         name=f"osb{par}{nh}")
                        nc.vector.tensor_copy(osb[:], psum_out[nh])
                        nc.scalar.dma_start(out_d[nh * 8:(nh + 1) * 8, :],
                                            osb[:])
                else:
                    out_sb = wpool.tile([16, 5], F32, tag=f"out_sb{par}")
                    nc.vector.memset(out_sb[:], 0.0)
                    nc.scalar.dma_start(out_d[:], out_sb[:])

            if timing_loop is None:
                emit(0)
            else:
                krep = int(_os.environ.get("KREP", "2"))
                kw = {}
                if _os.environ.get("KSTAG", "0") == "1":
                    kw["staggered_reset"] = True
                if _os.environ.get("KHINT", "0") == "1":
                    kw["hint_engines"] = (mybir.EngineType.PE,
                                          mybir.EngineType.SP,
                                          mybir.EngineType.DVE,
                                          mybir.EngineType.Activation)
                with tc.For_i(0, timing_loop, 1, **kw):
                    for r in range(krep):
                        emit(r % 2)

    nc.compile()
    return nc


_NC = {}


def _get_nc(timing_loop=None):
    if timing_loop not in _NC:
        _NC[timing_loop] = _build_nc(timing_loop)
    return _NC[timing_loop]


def _make_consts(conv_dct_w, conv_dct_b, bn_dct_g, bn_dct_b,
                 conv_grad_w, conv_grad_b, bn_grad_g, bn_grad_b,
                 fus_w, cls_w):
    A = build_A()
    AT = np.ascontiguousarray(A.T)  # [256, 8]
    consts = {}
    at = np.zeros((512, 8), np.float32)
    at[0:128] = AT[0::2]
    at[128:256] = AT[1::2]
    at[256:384] = AT[0:128]
    at[384:512] = AT[128:256]
    consts["at"] = at.astype(NP_DT16)

    BN_EPS = 1e-5
    convw = np.zeros((3, 1152), np.float32)
    bias = np.zeros((128, 1), np.float32)
    for br, (w, b, g, beta) in enumerate((
            (conv_dct_w, conv_dct_b, bn_dct_g, bn_dct_b),
            (conv_grad_w, conv_grad_b, bn_grad_g, bn_grad_b))):
        g_eff = (g / np.sqrt(1.0 + BN_EPS)).astype(np.float32)
        w_eff = w * g_eff[:, None, None, None]
        if br == 1:
            w_eff = w_eff * 0.25  # fold the 4-sample average
        b_eff = b * g_eff + beta
        for di in range(3):
            for dj in range(3):
                # convw[ic, (br*9+di*3+dj)*64 + oc] = w_eff[oc, ic, di, dj]
                off = (br * 9 + di * 3 + dj) * 64
                convw[:, off:off + 64] = w_eff[:, :, di, dj].T
        bias[br * 64:(br + 1) * 64, 0] = b_eff
    consts["convw"] = convw.astype(NP_DT16)
    consts["bias"] = bias

    ccls = np.zeros((128, 128), np.float32)
    for k in range(2):
        ccls[0:64, k * 64:(k + 1) * 64] = cls_w[k].reshape(64, 64)
        ccls[64:128, k * 64:(k + 1) * 64] = cls_w[k].reshape(64, 64)
    consts["ccls"] = ccls.astype(NP_DT16)
    consts["fusw"] = np.ascontiguousarray(np.tile(fus_w[0][:, None] / 64.0, (2, 1)))
    sel = np.zeros((128, 2), np.float32)
    sel[0:64, 0] = 1.0
    sel[64:128, 1] = 1.0
    consts["sel"] = sel
    return consts


def kernel_with_results(x, conv_dct_w, conv_dct_b, bn_dct_g, bn_dct_b,
                        conv_grad_w, conv_grad_b, bn_grad_g, bn_grad_b,
                        fus_w, fus_b, cls_w, cls_b, trace=False):
    nc = _get_nc()
    consts = _make_consts(conv_dct_w, conv_dct_b, bn_dct_g, bn_dct_b,
                          conv_grad_w, conv_grad_b, bn_grad_g, bn_grad_b,
                          fus_w, cls_w)
    x = np.ascontiguousarray(np.asarray(x, np.float32))
    in_maps = []
    for i in range(N_CORES):
        m = {"x": np.ascontiguousarray(x[i * B:(i + 1) * B])}
        m.update(consts)
        in_maps.append(m)
    res = run_bass_kernel_spmd(nc, in_maps, list(range(N_CORES)), trace=trace)

    outs = []
    for i in range(N_CORES):
        r = res.results[i]["out"]  # [16, 5]
        Pd = r[:, [0, 2]]
        Pg = r[:, [1, 3]]
        sl = r[:, 4] + np.float32(fus_b[0])
        w = 1.0 / (1.0 + np.exp(-sl))[:, None]
        outs.append(w * Pd + (1.0 - w) * Pg + np.asarray(cls_b)[None, :])
    return np.concatenate(outs, axis=0).astype(np.float32), res


def kernel(**inputs):
    out, _ = kernel_with_results(**inputs)
    return out


# revision 16
# speedup vs baseline: 2.6058x; 1.3487x over previous
"""Trainium2 Bass kernel for nn_AdaptiveFusionNet (8-core data parallel).

Math collapse (validated vs reference):
  - DCT branch: apply_dct(x)[b,c] == A @ X @ A.T with A = R @ D[:,:25] @ D[:25,:]
    (R = 256->8 bilinear resize matrix; note reference's "inverse" einsum
    applies D again, not D.T).
  - Gradient branch: bilinear 256->8 resize samples grad magnitude only at
    rows/cols {15,16}+32g with weight 1/4 each; sobel there needs only the
    4x4 neighborhoods {14..17}+32g (wrap never triggered). So only 32x32
    samples per channel are needed.
  - conv+BN+ReLU folded: w_eff = w*gamma/sqrt(1+eps), b_eff = b*g+beta.
  - Final: out = w*Pd + (1-w)*Pg + cls_b computed on host from per-core
    [16,5] device outputs (Pd0,Pg0,Pd1,Pg1,slogit).

v3 (from v1 trace analysis: fp32 MMs lower to 2 HW passes with ~205ns
128-col weight loads; stage1 was ~82us of PE):
  - whole DCT/conv pipeline in fp16 (FWL weight loads ~52ns, 1-pass MMs,
    DVE 2x); x DMAed f32 then cast per image alternating DVE/ACT.
  - x DMA uses 2-rows-per-partition layout (2KB descriptor runs, half
    the descriptors of the 1KB row-interleave layout); A is split by row
    parity to match.
  - stage2 operand swap: lhsT=w1t_sb rhs=A-half puts (b,i) on psum
    partitions directly, eliminating the PE transposes of v1.
  - PSUM: stage1's six [128,128] groups packed into 2 banks per loop
    parity (4+2; psum2T and the out matmuls reuse spare columns), conv
    psc uses 4 shared banks -> 8 banks total, so consecutive unrolled
    emits (KREP=2, parity-alternating) can overlap.
  - dconv/gconv are persistent pre-zeroed tiles (borders memset once).
  - sobel computed in fp16 (gpatch cast on the ACT column-sample copy).
  - DMA issue spread: xt on SyncE, gp/out on ScalarE, dflat/gflat on
    GpSimd (SWDGE).
"""
import sys

import numpy as np

try:
    import concourse  # noqa: F401
except ImportError:
    sys.path.insert(0, "/opt/trn_rl_repo")

import concourse.bass as bass
import concourse.bacc as bacc
import concourse.mybir as mybir
from concourse import tile
from concourse.bass_utils import run_bass_kernel_spmd

F32 = mybir.dt.float32
import os as _os_dt
DT16 = (mybir.dt.float16 if _os_dt.environ.get("KDT16", "fp16") == "fp16"
        else mybir.dt.bfloat16)
NP_DT16 = mybir.dt.np(DT16)
N_CORES = 8
B_TOTAL = 128
B = B_TOTAL // N_CORES  # 16 images per core
NCH = B * 3             # 48 channels per core


def build_A():
    N = 256
    n = np.arange(N, dtype=np.float64)
    k = n[:, None]
    D = np.cos(np.pi * (2.0 * n + 1.0) * k / (2.0 * N))
    scale = np.full((N, 1), np.sqrt(2.0 / N))
    scale[0, 0] = np.sqrt(1.0 / N)
    D = D * scale
    R = np.zeros((8, 256))
    for i in range(8):
        R[i, 15 + 32 * i] = 0.5
        R[i, 16 + 32 * i] = 0.5
    A = R @ D[:, :25] @ D[:25, :]
    return A.astype(np.float32)


def _build_nc(timing_loop=None):
    import os as _os
    drop = set(_os.environ.get("KDROP", "").split(","))
    nc = bacc.Bacc("TRN2", target_bir_lowering=False, debug=False,
                   num_devices=N_CORES)

    if timing_loop is None:
        x_d = nc.dram_tensor("x", [B, 3, 256, 256], F32, kind="ExternalInput")
    else:
        x_d = nc.dram_tensor("xint", [B, 3, 256, 256], F32)
    # at: rows 0:128 = A.T[0::2] (even src rows), 128:256 = A.T[1::2],
    #     256:384 = A.T[0:128], 384:512 = A.T[128:256] (stage2 halves)
    at_d = nc.dram_tensor("at", [512, 8], DT16, kind="ExternalInput")
    convw_d = nc.dram_tensor("convw", [3, 1152], DT16, kind="ExternalInput")
    bias_d = nc.dram_tensor("bias", [128, 1], F32, kind="ExternalInput")
    ccls_d = nc.dram_tensor("ccls", [128, 128], DT16, kind="ExternalInput")
    fusw_d = nc.dram_tensor("fusw", [128, 1], F32, kind="ExternalInput")
    sel_d = nc.dram_tensor("sel", [128, 2], F32, kind="ExternalInput")
    out_d = nc.dram_tensor("out", [B, 5], F32, kind="ExternalOutput")

    with tile.TileContext(nc) as tc:
        with (
            tc.tile_pool(name="const", bufs=1) as cpool,
            tc.tile_pool(name="xin", bufs=6) as xpool,
            tc.tile_pool(name="xbf", bufs=8) as xbpool,
            tc.tile_pool(name="work", bufs=1) as wpool,
            tc.tile_pool(name="scratch", bufs=2) as spool,
            tc.tile_pool(name="ps1", bufs=1, space="PSUM") as ps1,
            tc.tile_pool(name="ps2", bufs=1, space="PSUM") as ps2,
        ):
            # ---- constants ----
            a1e = cpool.tile([128, 8], DT16, tag="a1e")
            a1o = cpool.tile([128, 8], DT16, tag="a1o")
            a1c = cpool.tile([128, 8], DT16, tag="a1c")
            a2c = cpool.tile([128, 8], DT16, tag="a2c")
            nc.sync.dma_start(a1e[:], at_d[0:128, :])
            nc.sync.dma_start(a1o[:], at_d[128:256, :])
            nc.sync.dma_start(a1c[:], at_d[256:384, :])
            nc.sync.dma_start(a2c[:], at_d[384:512, :])
            convw = cpool.tile([3, 1152], DT16, tag="convw")
            nc.sync.dma_start(convw[:], convw_d[:])
            bias128 = cpool.tile([128, 1], F32, tag="bias128")
            nc.sync.dma_start(bias128[:], bias_d[:])
            ccls = cpool.tile([128, 128], DT16, tag="ccls")
            nc.sync.dma_start(ccls[:], ccls_d[:])
            fusw = cpool.tile([128, 1], F32, tag="fusw")
            nc.sync.dma_start(fusw[:], fusw_d[:])
            sel = cpool.tile([128, 2], F32, tag="sel")
            nc.sync.dma_start(sel[:], sel_d[:])
            # persistent zero-bordered conv inputs, one pair per parity
            convin = [[cpool.tile([3, B, 10, 10], DT16, tag=f"cvin{p}_{br}",
                                  name=f"cvin{p}_{br}")
                       for br in range(2)] for p in range(2)]
            for p in range(2):
                for br in range(2):
                    nc.vector.memset(convin[p][br][:], 0.0)

            SUB = mybir.AluOpType.subtract
            ADD = mybir.AluOpType.add
            MUL = mybir.AluOpType.mult

            kxsw = _os.environ.get("KXSW", "1") == "1"

            def emit(par):
                # ---- PSUM banks for this parity ----
                bankA = ps1.tile([128, 512], F32, tag=f"psA{par}",
                                 name=f"bankA{par}")
                bankB = ps1.tile([128, 512], F32, tag=f"psB{par}",
                                 name=f"bankB{par}")

                def w1t_view(ic, cb):
                    if ic < 2:
                        off = (ic * 2 + cb) * 128
                        return bankA[:, off:off + 128]
                    return bankB[:, cb * 128:cb * 128 + 128]

                # ---- stage 1: X^T A^T per channel (fp16) ----
                for b in range(B):
                    xtb = xbpool.tile([128, 3, 2, 256], DT16, tag="xtb")
                    xv = x_d[b].rearrange("c (p r2) w -> p c r2 w", r2=2)
                    if kxsw:
                        nc.gpsimd.dma_start(xtb[:], xv)  # SWDGE casts f32->f16
                    else:
                        xt = xpool.tile([128, 3, 2, 256], F32, tag="xt")
                        nc.sync.dma_start(xt[:], xv)
                        if b % 3 == 2:
                            nc.scalar.copy(xtb[:], xt[:])
                        else:
                            nc.vector.tensor_copy(xtb[:], xt[:])
                    for ic in range(3):
                        for cb in range(2):
                            for r2 in range(2):
                                nc.tensor.matmul(
                                    w1t_view(ic, cb)[:, b * 8:(b + 1) * 8],
                                    lhsT=xtb[:, ic, r2, cb * 128:(cb + 1) * 128],
                                    rhs=(a1e[:] if r2 == 0 else a1o[:]),
                                    start=(r2 == 0), stop=(r2 == 1))

                # ---- gradient branch: row gather DMAs (ic-major partitions) ----
                gp = wpool.tile([NCH, 8, 4, 256], DT16, tag=f"gp{par}")
                if "gpdma" not in drop:
                    for ic in range(3):
                        src = x_d[:, ic].rearrange(
                            "b (g h) w -> b g h w", h=32)[:, :, 14:18, :]
                        nc.gpsimd.dma_start(gp[ic * B:(ic + 1) * B], src)

                # ---- stage 2 (swapped): psum2T[(b,i), ic*8+j] ----
                dcttail = "dcttail" not in drop
                w1t_sb = [[wpool.tile([128, 128], DT16,
                                      tag=f"w1sb{par}_{ic}_{cb}",
                                      name=f"w1sb{par}_{ic}_{cb}")
                           for cb in range(2)] for ic in range(3)]
                if dcttail:
                    for ic in range(3):
                        for cb in range(2):
                            nc.vector.tensor_copy(w1t_sb[ic][cb][:],
                                                  w1t_view(ic, cb))
                psum2T = bankB[:, 256:280]  # [128, 24] in bankB spare cols
                dt_sb = wpool.tile([128, 24], DT16, tag=f"dt_sb{par}")
                dflat = wpool.tile([3, 1024], DT16, tag=f"dflat{par}")
                dconv = convin[par][0]
                if dcttail:
                    for ic in range(3):
                        for cb in range(2):
                            nc.tensor.matmul(
                                psum2T[:, ic * 8:(ic + 1) * 8],
                                lhsT=w1t_sb[ic][cb][:],
                                rhs=(a1c[:] if cb == 0 else a2c[:]),
                                start=(cb == 0), stop=(cb == 1))
                    nc.vector.tensor_copy(dt_sb[:], psum2T)
                    for ic in range(3):
                        nc.sync.dma_start(dflat[ic:ic + 1, :],
                                          dt_sb[:, ic * 8:(ic + 1) * 8])
                    nc.vector.tensor_copy(
                        dconv[:, :, 1:9, 1:9],
                        dflat.rearrange("c (b i j) -> c b i j", b=B, i=8))

                # ---- gradient branch: col sample + sobel (fp16) ----
                gpatch = wpool.tile([NCH, 8, 4, 8, 4], DT16, tag=f"gpatch{par}")
                g8 = wpool.tile([NCH, 8, 8, 1], DT16, tag=f"g8{par}")
                gflat = wpool.tile([3, 1024], DT16, tag=f"gflat{par}")
                gconv = convin[par][1]
                if "sobel" not in drop:
                    gp5 = gp.rearrange("p g r (gc c) -> p g r gc c", c=32)
                    for g in range(8):
                        nc.scalar.copy(gpatch[:, g], gp5[:, g, :, :, 14:18])

                    def PP(r, dc):
                        return gpatch[:, :, r, :, 1 + dc:3 + dc]  # [48, 8, 8, 2]

                    a_lr = []
                    for lr in (1, 2):
                        t1 = spool.tile([NCH, 8, 8, 2], DT16, tag="t1")
                        t2 = spool.tile([NCH, 8, 8, 2], DT16, tag="t2")
                        t3 = spool.tile([NCH, 8, 8, 2], DT16, tag="t3")
                        nc.vector.tensor_tensor(t1[:], PP(lr - 1, 1), PP(lr - 1, -1), SUB)
                        nc.vector.tensor_tensor(t2[:], PP(lr, 1), PP(lr, -1), SUB)
                        nc.vector.tensor_tensor(t3[:], PP(lr + 1, 1), PP(lr + 1, -1), SUB)
                        u = spool.tile([NCH, 8, 8, 2], DT16, tag="u")
                        nc.vector.tensor_tensor(u[:], t1[:], t3[:], ADD)
                        gx = spool.tile([NCH, 8, 8, 2], DT16, tag="gx")
                        nc.vector.scalar_tensor_tensor(gx[:], t2[:], 2.0, u[:], MUL, ADD)
                        s1 = spool.tile([NCH, 8, 8, 2], DT16, tag="s1")
                        s2 = spool.tile([NCH, 8, 8, 2], DT16, tag="s2")
                        s3 = spool.tile([NCH, 8, 8, 2], DT16, tag="s3")
                        nc.vector.tensor_tensor(s1[:], PP(lr + 1, -1), PP(lr - 1, -1), SUB)
                        nc.vector.tensor_tensor(s2[:], PP(lr + 1, 0), PP(lr - 1, 0), SUB)
                        nc.vector.tensor_tensor(s3[:], PP(lr + 1, 1), PP(lr - 1, 1), SUB)
                        u2 = spool.tile([NCH, 8, 8, 2], DT16, tag="u2")
                        nc.vector.tensor_tensor(u2[:], s1[:], s3[:], ADD)
                        gy = spool.tile([NCH, 8, 8, 2], DT16, tag="gy")
                        nc.vector.scalar_tensor_tensor(gy[:], s2[:], 2.0, u2[:], MUL, ADD)
                        nc.vector.tensor_tensor(gx[:], gx[:], gx[:], MUL)
                        nc.vector.tensor_tensor(gy[:], gy[:], gy[:], MUL)
                        m2 = spool.tile([NCH, 8, 8, 2], DT16, tag="m2")
                        nc.vector.tensor_tensor(m2[:], gx[:], gy[:], ADD)
                        mag = spool.tile([NCH, 8, 8, 2], DT16, tag=f"mag{lr}")
                        nc.scalar.sqrt(mag[:], m2[:])
                        al = spool.tile([NCH, 8, 8, 1], DT16, tag=f"al{lr}")
                        nc.vector.tensor_tensor(al[:], mag[:, :, :, 0:1], mag[:, :, :, 1:2], ADD)
                        a_lr.append(al)

                    nc.vector.tensor_tensor(g8[:], a_lr[0][:], a_lr[1][:], ADD)
                    g8f = g8.rearrange("p a b c -> p (a b c)")  # [48, 64]

                    for ic in range(3):
                        nc.sync.dma_start(gflat[ic:ic + 1, :],
                                          g8f[ic * B:(ic + 1) * B, :])
                    nc.vector.tensor_copy(
                        gconv[:, :, 1:9, 1:9],
                        gflat.rearrange("c (b i j) -> c b i j", b=B, i=8))

                # ---- convs (fp16, br packed on array col-groups) ----
                psc = [ps1.tile([128, 512], F32, tag=f"psc{par}_{nh}",
                                name=f"psc{par}_{nh}") for nh in range(2)]
                if not ("conv" in drop or "tail" in drop):
                    for nh in range(2):
                        for si, (di, dj) in enumerate(
                                (di, dj) for di in range(3) for dj in range(3)):
                            for br, rhs_t in ((0, dconv), (1, gconv)):
                                w_off = (br * 9 + di * 3 + dj) * 64
                                rv = rhs_t[:, nh * 8:(nh + 1) * 8,
                                           di:di + 8, dj:dj + 8]
                                nc.tensor.matmul(
                                    psc[nh][br * 64:(br + 1) * 64, :],
                                    lhsT=convw[:, w_off:w_off + 64],
                                    rhs=rv,
                                    start=(si == 0), stop=(si == 8))

                # relu(conv + bias) -> dg_sb [128, 1024] fp16
                tail_on = "tail" not in drop
                dg_sb = wpool.tile([128, 1024], DT16, tag=f"dg_sb{par}")
                if tail_on:
                    for nh in range(2):
                        nc.scalar.activation(
                            dg_sb[:, nh * 512:(nh + 1) * 512],
                            psc[nh][:],
                            mybir.ActivationFunctionType.Relu,
                            bias=bias128[:],
                            scale=1.0)

                    # ---- fusion + classifier (per half for pipelining) ----
                    HB = B // 2
                    psum_out = []
                    for nh in range(2):
                        dgh = dg_sb[:, nh * 512:(nh + 1) * 512].rearrange(
                            "p (b f) -> p b f", b=HB)
                        s_red = wpool.tile([128, HB, 1], F32,
                                           tag=f"s_red{par}{nh}",
                                           name=f"s_red{par}{nh}")
                        nc.vector.reduce_sum(s_red[:], dgh,
                                             axis=mybir.AxisListType.X)
                        tk_red = []
                        for k in range(2):
                            tmpk = spool.tile([128, HB, 64], DT16, tag="tmpk")
                            cc = ccls[:, k * 64:(k + 1) * 64].unsqueeze(1)
                            nc.vector.tensor_tensor(
                                tmpk[:], dgh, cc.broadcast_to([128, HB, 64]), MUL)
                            tkr = wpool.tile([128, HB, 1], F32,
                                             tag=f"tkr{par}_{nh}_{k}",
                                             name=f"tkr{par}_{nh}_{k}")
                            nc.vector.reduce_sum(tkr[:], tmpk[:],
                                                 axis=mybir.AxisListType.X)
                            tk_red.append(tkr)
                        po = bankB[0:HB, 288 + 16 * nh:293 + 16 * nh]
                        for k in range(2):
                            nc.tensor.matmul(po[0:HB, 2 * k:2 * k + 2],
                                             lhsT=tk_red[k][:], rhs=sel[:],
                                             start=True, stop=True)
                        nc.tensor.matmul(po[0:HB, 4:5], lhsT=s_red[:],
                                         rhs=fusw[:], start=True, stop=True)
                        psum_out.append(po)

                if tail_on:
                    for nh in range(2):
                        osb = wpool.tile([8, 5], F32, tag=f"osb{par}{nh}",
                                         name=f"osb{par}{nh}")
                        nc.vector.tensor_copy(osb[:], psum_out[nh])
                        nc.scalar.dma_start(out_d[nh * 8:(nh + 1) * 8, :],
                                            osb[:])
                else:
                    out_sb = wpool.tile([16, 5], F32, tag=f"out_sb{par}")
                    nc.vector.memset(out_sb[:], 0.0)
                    nc.scalar.dma_start(out_d[:], out_sb[:])

            if timing_loop is None:
                emit(0)
            else:
                krep = int(_os.environ.get("KREP", "2"))
                kw = {}
                if _os.environ.get("KSTAG", "0") == "1":
                    kw["staggered_reset"] = True
                if _os.environ.get("KHINT", "0") == "1":
                    kw["hint_engines"] = (mybir.EngineType.PE,
                                          mybir.EngineType.SP,
                                          mybir.EngineType.DVE,
                                          mybir.EngineType.Activation)
                with tc.For_i(0, timing_loop, 1, **kw):
                    for r in range(krep):
                        emit(r % 2)

    nc.compile()
    return nc


_NC = {}


def _get_nc(timing_loop=None):
    if timing_loop not in _NC:
        _NC[timing_loop] = _build_nc(timing_loop)
    return _NC[timing_loop]


def _make_consts(conv_dct_w, conv_dct_b, bn_dct_g, bn_dct_b,
                 conv_grad_w, conv_grad_b, bn_grad_g, bn_grad_b,
                 fus_w, cls_w):
    A = build_A()
    AT = np.ascontiguousarray(A.T)  # [256, 8]
    consts = {}
    at = np.zeros((512, 8), np.float32)
    at[0:128] = AT[0::2]
    at[128:256] = AT[1::2]
    at[256:384] = AT[0:128]
    at[384:512] = AT[128:256]
    consts["at"] = at.astype(NP_DT16)

    BN_EPS = 1e-5
    convw = np.zeros((3, 1152), np.float32)
    bias = np.zeros((128, 1), np.float32)
    for br, (w, b, g, beta) in enumerate((
            (conv_dct_w, conv_dct_b, bn_dct_g, bn_dct_b),
            (conv_grad_w, conv_grad_b, bn_grad_g, bn_grad_b))):
        g_eff = (g / np.sqrt(1.0 + BN_EPS)).astype(np.float32)
        w_eff = w * g_eff[:, None, None, None]
        if br == 1:
            w_eff = w_eff * 0.25  # fold the 4-sample average
        b_eff = b * g_eff + beta
        for di in range(3):
            for dj in range(3):
                # convw[ic, (br*9+di*3+dj)*64 + oc] = w_eff[oc, ic, di, dj]
                off = (br * 9 + di * 3 + dj) * 64
                convw[:, off:off + 64] = w_eff[:, :, di, dj].T
        bias[br * 64:(br + 1) * 64, 0] = b_eff
    consts["convw"] = convw.astype(NP_DT16)
    consts["bias"] = bias

    ccls = np.zeros((128, 128), np.float32)
    for k in range(2):
        ccls[0:64, k * 64:(k + 1) * 64] = cls_w[k].reshape(64, 64)
        ccls[64:128, k * 64:(k + 1) * 64] = cls_w[k].reshape(64, 64)
    consts["ccls"] = ccls.astype(NP_DT16)
    consts["fusw"] = np.ascontiguousarray(np.tile(fus_w[0][:, None] / 64.0, (2, 1)))
    sel = np.zeros((128, 2), np.float32)
    sel[0:64, 0] = 1.0
    sel[64:128, 1] = 1.0
    consts["sel"] = sel
    return consts


def kernel_with_results(x, conv_dct_w, conv_dct_b, bn_dct_g, bn_dct_b,
                        conv_grad_w, conv_grad_b, bn_grad_g, bn_grad_b,
                        fus_w, fus_b, cls_w, cls_b, trace=False):
    nc = _get_nc()
    consts = _make_consts(conv_dct_w, conv_dct_b, bn_dct_g, bn_dct_b,
                          conv_grad_w, conv_grad_b, bn_grad_g, bn_grad_b,
                          fus_w, cls_w)
    x = np.ascontiguousarray(np.asarray(x, np.float32))
    in_maps = []
    for i in range(N_CORES):
        m = {"x": np.ascontiguousarray(x[i * B:(i + 1) * B])}
        m.update(consts)
        in_maps.append(m)
    res = run_bass_kernel_spmd(nc, in_maps, list(range(N_CORES)), trace=trace)

    outs = []
    for i in range(N_CORES):
        r = res.results[i]["out"]  # [16, 5]
        Pd = r[:, [0, 2]]
        Pg = r[:, [1, 3]]
        sl = r[:, 4] + np.float32(fus_b[0])
        w = 1.0 / (1.0 + np.exp(-sl))[:, None]
        outs.append(w * Pd + (1.0 - w) * Pg + np.asarray(cls_b)[None, :])
    return np.concatenate(outs, axis=0).astype(np.float32), res


def kernel(**inputs):
    out, _ = kernel_with_results(**inputs)
    return out


# revision 22
# speedup vs baseline: 2.6252x; 1.0074x over previous
"""Trainium2 Bass kernel for nn_AdaptiveFusionNet (8-core data parallel).

Math collapse (validated vs reference):
  - DCT branch: apply_dct(x)[b,c] == A @ X @ A.T with A = R @ D[:,:25] @ D[:25,:]
    (R = 256->8 bilinear resize matrix; note reference's "inverse" einsum
    applies D again, not D.T).
  - Gradient branch: bilinear 256->8 resize samples grad magnitude only at
    rows/cols {15,16}+32g with weight 1/4 each; sobel there needs only the
    4x4 neighborhoods {14..17}+32g (wrap never triggered). So only 32x32
    samples per channel are needed.
  - conv+BN+ReLU folded: w_eff = w*gamma/sqrt(1+eps), b_eff = b*g+beta.
  - Final: out = w*Pd + (1-w)*Pg + cls_b computed on host from per-core
    [16,5] device outputs (Pd0,Pg0,Pd1,Pg1,slogit).

v5 final (evolved via trace analysis; loop-amortized ~47us/batch vs
124us baseline, ~42us DMA roofline for the 14.2MB/core of HBM reads):
  - v1 was PE-bound: fp32 matmuls lower to 2 HW passes with ~205ns
    128-col weight loads each (stage1 = 192 fp32 MMs = ~82us of PE).
    Whole DCT/conv pipeline now runs fp16 (FWL ~52ns weight loads,
    1-pass MMs, DVE 2x). fp16 (not bf16: 3.5e-2 rel err, fails 2e-2).
  - x loaded by SWDGE (gpsimd) DMAs that cast f32->f16 in-flight; no
    engine cast stage. 2-rows-per-partition layout (2KB descriptor
    runs); stage1 A-matrix split by row parity to match. gp row-gather
    also SWDGE-cast. (Keeping both on the one SWDGE queue matters:
    moving gp to HWDGE/sync regressed 47->67us.)
  - stage2 operand swap: lhsT=w1t_sb, rhs=A-half puts (b,i) on psum
    partitions directly, eliminating v1's 3 PE transposes + 2 copies.
  - PSUM plan (8 banks): per loop-parity 2 banks hold stage1's six
    [128,128] accum groups (4+2; psum2T [128,24] and the tiny out
    matmuls live in bankB's spare columns); conv psc = 2 banks per
    parity with the two branches packed on array col-groups 0-63/64-127
    (concurrent MMs via auto tile_position, halves conv span; single
    relu per half with the stacked 128-bias).
  - dconv/gconv are persistent pre-zeroed tiles (borders memset once,
    outside the loop). sobel in fp16 (gpatch cast on ACT column-sample
    copies). dflat/gflat SBUF-SBUF shuffles on SyncE.
  - timing loop: KREP=8 emits per For_i iteration, parity-alternating,
    with branch-prefetch hints (KHINT); measured time divides by KREP.
    Staggered reset measured worse at every config; full-barrier
    back-edge amortized over 8 emits instead.
"""
import sys

import numpy as np

try:
    import concourse  # noqa: F401
except ImportError:
    sys.path.insert(0, "/opt/trn_rl_repo")

import concourse.bass as bass
import concourse.bacc as bacc
import concourse.mybir as mybir
from concourse import tile
from concourse.bass_utils import run_bass_kernel_spmd

F32 = mybir.dt.float32
import os as _os_dt
DT16 = (mybir.dt.float16 if _os_dt.environ.get("KDT16", "fp16") == "fp16"
        else mybir.dt.bfloat16)
NP_DT16 = mybir.dt.np(DT16)
N_CORES = 8
B_TOTAL = 128
B = B_TOTAL // N_CORES  # 16 images per core
NCH = B * 3             # 48 channels per core


def build_A():
    N = 256
    n = np.arange(N, dtype=np.float64)
    k = n[:, None]
    D = np.cos(np.pi * (2.0 * n + 1.0) * k / (2.0 * N))
    scale = np.full((N, 1), np.sqrt(2.0 / N))
    scale[0, 0] = np.sqrt(1.0 / N)
    D = D * scale
    R = np.zeros((8, 256))
    for i in range(8):
        R[i, 15 + 32 * i] = 0.5
        R[i, 16 + 32 * i] = 0.5
    A = R @ D[:, :25] @ D[:25, :]
    return A.astype(np.float32)


def _build_nc(timing_loop=None):
    import os as _os
    drop = set(_os.environ.get("KDROP", "").split(","))
    nc = bacc.Bacc("TRN2", target_bir_lowering=False, debug=False,
                   num_devices=N_CORES)

    if timing_loop is None:
        x_d = nc.dram_tensor("x", [B, 3, 256, 256], F32, kind="ExternalInput")
    else:
        x_d = nc.dram_tensor("xint", [B, 3, 256, 256], F32)
    # at: rows 0:128 = A.T[0::2] (even src rows), 128:256 = A.T[1::2],
    #     256:384 = A.T[0:128], 384:512 = A.T[128:256] (stage2 halves)
    at_d = nc.dram_tensor("at", [512, 8], DT16, kind="ExternalInput")
    convw_d = nc.dram_tensor("convw", [3, 1152], DT16, kind="ExternalInput")
    bias_d = nc.dram_tensor("bias", [128, 1], F32, kind="ExternalInput")
    ccls_d = nc.dram_tensor("ccls", [128, 128], DT16, kind="ExternalInput")
    fusw_d = nc.dram_tensor("fusw", [128, 1], F32, kind="ExternalInput")
    sel_d = nc.dram_tensor("sel", [128, 2], F32, kind="ExternalInput")
    out_d = nc.dram_tensor("out", [B, 5], F32, kind="ExternalOutput")

    with tile.TileContext(nc) as tc:
        with (
            tc.tile_pool(name="const", bufs=1) as cpool,
            tc.tile_pool(name="xin", bufs=6) as xpool,
            tc.tile_pool(name="xbf", bufs=8) as xbpool,
            tc.tile_pool(name="work", bufs=1) as wpool,
            tc.tile_pool(name="scratch", bufs=2) as spool,
            tc.tile_pool(name="ps1", bufs=1, space="PSUM") as ps1,
            tc.tile_pool(name="ps2", bufs=1, space="PSUM") as ps2,
        ):
            # ---- constants ----
            a1e = cpool.tile([128, 8], DT16, tag="a1e")
            a1o = cpool.tile([128, 8], DT16, tag="a1o")
            a1c = cpool.tile([128, 8], DT16, tag="a1c")
            a2c = cpool.tile([128, 8], DT16, tag="a2c")
            nc.sync.dma_start(a1e[:], at_d[0:128, :])
            nc.sync.dma_start(a1o[:], at_d[128:256, :])
            nc.sync.dma_start(a1c[:], at_d[256:384, :])
            nc.sync.dma_start(a2c[:], at_d[384:512, :])
            convw = cpool.tile([3, 1152], DT16, tag="convw")
            nc.sync.dma_start(convw[:], convw_d[:])
            bias128 = cpool.tile([128, 1], F32, tag="bias128")
            nc.sync.dma_start(bias128[:], bias_d[:])
            ccls = cpool.tile([128, 128], DT16, tag="ccls")
            nc.sync.dma_start(ccls[:], ccls_d[:])
            fusw = cpool.tile([128, 1], F32, tag="fusw")
            nc.sync.dma_start(fusw[:], fusw_d[:])
            sel = cpool.tile([128, 2], F32, tag="sel")
            nc.sync.dma_start(sel[:], sel_d[:])
            # persistent zero-bordered conv inputs, one pair per parity
            convin = [[cpool.tile([3, B, 10, 10], DT16, tag=f"cvin{p}_{br}",
                                  name=f"cvin{p}_{br}")
                       for br in range(2)] for p in range(2)]
            for p in range(2):
                for br in range(2):
                    nc.vector.memset(convin[p][br][:], 0.0)

            SUB = mybir.AluOpType.subtract
            ADD = mybir.AluOpType.add
            MUL = mybir.AluOpType.mult

            kxsw = int(_os.environ.get("KXSW", "1"))

            def emit(par):
                # ---- PSUM banks for this parity ----
                bankA = ps1.tile([128, 512], F32, tag=f"psA{par}",
                                 name=f"bankA{par}")
                bankB = ps1.tile([128, 512], F32, tag=f"psB{par}",
                                 name=f"bankB{par}")

                def w1t_view(ic, cb):
                    if ic < 2:
                        off = (ic * 2 + cb) * 128
                        return bankA[:, off:off + 128]
                    return bankB[:, cb * 128:cb * 128 + 128]

                # ---- stage 1: X^T A^T per channel (fp16) ----
                for b in range(B):
                    xtb = xbpool.tile([128, 3, 2, 256], DT16, tag="xtb")
                    xv = x_d[b].rearrange("c (p r2) w -> p c r2 w", r2=2)
                    swdge_this = kxsw == 1 or (kxsw == 2 and b % 2 == 0)
                    if swdge_this:
                        nc.gpsimd.dma_start(xtb[:], xv)  # SWDGE casts f32->f16
                    else:
                        xt = xpool.tile([128, 3, 2, 256], F32, tag="xt")
                        nc.sync.dma_start(xt[:], xv)
                        if b % 4 == 3:
                            nc.scalar.copy(xtb[:], xt[:])
                        else:
                            nc.vector.tensor_copy(xtb[:], xt[:])
                    for ic in range(3):
                        for cb in range(2):
                            for r2 in range(2):
                                nc.tensor.matmul(
                                    w1t_view(ic, cb)[:, b * 8:(b + 1) * 8],
                                    lhsT=xtb[:, ic, r2, cb * 128:(cb + 1) * 128],
                                    rhs=(a1e[:] if r2 == 0 else a1o[:]),
                                    start=(r2 == 0), stop=(r2 == 1))

                # ---- gradient branch: row gather DMAs (ic-major partitions) ----
                gp = wpool.tile([NCH, 8, 4, 256], DT16, tag=f"gp{par}")
                if "gpdma" not in drop:
                    for ic in range(3):
                        src = x_d[:, ic].rearrange(
                            "b (g h) w -> b g h w", h=32)[:, :, 14:18, :]
                        nc.gpsimd.dma_start(gp[ic * B:(ic + 1) * B], src)

                # ---- stage 2 (swapped): psum2T[(b,i), ic*8+j] ----
                dcttail = "dcttail" not in drop
                w1t_sb = [[wpool.tile([128, 128], DT16,
                                      tag=f"w1sb{par}_{ic}_{cb}",
                                      name=f"w1sb{par}_{ic}_{cb}")
                           for cb in range(2)] for ic in range(3)]
                if dcttail:
                    for ic in range(3):
                        for cb in range(2):
                            nc.vector.tensor_copy(w1t_sb[ic][cb][:],
                                                  w1t_view(ic, cb))
                psum2T = bankB[:, 256:280]  # [128, 24] in bankB spare cols
                dt_sb = wpool.tile([128, 24], DT16, tag=f"dt_sb{par}")
                dflat = wpool.tile([3, 1024], DT16, tag=f"dflat{par}")
                dconv = convin[par][0]
                if dcttail:
                    for ic in range(3):
                        for cb in range(2):
                            nc.tensor.matmul(
                                psum2T[:, ic * 8:(ic + 1) * 8],
                                lhsT=w1t_sb[ic][cb][:],
                                rhs=(a1c[:] if cb == 0 else a2c[:]),
                                start=(cb == 0), stop=(cb == 1))
                    nc.vector.tensor_copy(dt_sb[:], psum2T)
                    for ic in range(3):
                        nc.sync.dma_start(dflat[ic:ic + 1, :],
                                          dt_sb[:, ic * 8:(ic + 1) * 8])
                    nc.vector.tensor_copy(
                        dconv[:, :, 1:9, 1:9],
                        dflat.rearrange("c (b i j) -> c b i j", b=B, i=8))

                # ---- gradient branch: col sample + sobel (fp16) ----
                gpatch = wpool.tile([NCH, 8, 4, 8, 4], DT16, tag=f"gpatch{par}")
                g8 = wpool.tile([NCH, 8, 8, 1], DT16, tag=f"g8{par}")
                gflat = wpool.tile([3, 1024], DT16, tag=f"gflat{par}")
                gconv = convin[par][1]
                if "sobel" not in drop:
                    gp5 = gp.rearrange("p g r (gc c) -> p g r gc c", c=32)
                    for g in range(8):
                        nc.scalar.copy(gpatch[:, g], gp5[:, g, :, :, 14:18])

                    def PP(r, dc):
                        return gpatch[:, :, r, :, 1 + dc:3 + dc]  # [48, 8, 8, 2]

                    a_lr = []
                    for lr in (1, 2):
                        t1 = spool.tile([NCH, 8, 8, 2], DT16, tag="t1")
                        t2 = spool.tile([NCH, 8, 8, 2], DT16, tag="t2")
                        t3 = spool.tile([NCH, 8, 8, 2], DT16, tag="t3")
                        nc.vector.tensor_tensor(t1[:], PP(lr - 1, 1), PP(lr - 1, -1), SUB)
                        nc.vector.tensor_tensor(t2[:], PP(lr, 1), PP(lr, -1), SUB)
                        nc.vector.tensor_tensor(t3[:], PP(lr + 1, 1), PP(lr + 1, -1), SUB)
                        u = spool.tile([NCH, 8, 8, 2], DT16, tag="u")
                        nc.vector.tensor_tensor(u[:], t1[:], t3[:], ADD)
                        gx = spool.tile([NCH, 8, 8, 2], DT16, tag="gx")
                        nc.vector.scalar_tensor_tensor(gx[:], t2[:], 2.0, u[:], MUL, ADD)
                        s1 = spool.tile([NCH, 8, 8, 2], DT16, tag="s1")
                        s2 = spool.tile([NCH, 8, 8, 2], DT16, tag="s2")
                        s3 = spool.tile([NCH, 8, 8, 2], DT16, tag="s3")
                        nc.vector.tensor_tensor(s1[:], PP(lr + 1, -1), PP(lr - 1, -1), SUB)
                        nc.vector.tensor_tensor(s2[:], PP(lr + 1, 0), PP(lr - 1, 0), SUB)
                        nc.vector.tensor_tensor(s3[:], PP(lr + 1, 1), PP(lr - 1, 1), SUB)
                        u2 = spool.tile([NCH, 8, 8, 2], DT16, tag="u2")
                        nc.vector.tensor_tensor(u2[:], s1[:], s3[:], ADD)
                        gy = spool.tile([NCH, 8, 8, 2], DT16, tag="gy")
                        nc.vector.scalar_tensor_tensor(gy[:], s2[:], 2.0, u2[:], MUL, ADD)
                        nc.vector.tensor_tensor(gx[:], gx[:], gx[:], MUL)
                        nc.vector.tensor_tensor(gy[:], gy[:], gy[:], MUL)
                        m2 = spool.tile([NCH, 8, 8, 2], DT16, tag="m2")
                        nc.vector.tensor_tensor(m2[:], gx[:], gy[:], ADD)
                        mag = spool.tile([NCH, 8, 8, 2], DT16, tag=f"mag{lr}")
                        nc.scalar.sqrt(mag[:], m2[:])
                        al = spool.tile([NCH, 8, 8, 1], DT16, tag=f"al{lr}")
                        nc.vector.tensor_tensor(al[:], mag[:, :, :, 0:1], mag[:, :, :, 1:2], ADD)
                        a_lr.append(al)

                    nc.vector.tensor_tensor(g8[:], a_lr[0][:], a_lr[1][:], ADD)
                    g8f = g8.rearrange("p a b c -> p (a b c)")  # [48, 64]

                    for ic in range(3):
                        nc.sync.dma_start(gflat[ic:ic + 1, :],
                                          g8f[ic * B:(ic + 1) * B, :])
                    nc.vector.tensor_copy(
                        gconv[:, :, 1:9, 1:9],
                        gflat.rearrange("c (b i j) -> c b i j", b=B, i=8))

                # ---- convs (fp16, br packed on array col-groups) ----
                psc = [ps1.tile([128, 512], F32, tag=f"psc{par}_{nh}",
                                name=f"psc{par}_{nh}") for nh in range(2)]
                if not ("conv" in drop or "tail" in drop):
                    for nh in range(2):
                        for si, (di, dj) in enumerate(
                                (di, dj) for di in range(3) for dj in range(3)):
                            for br, rhs_t in ((0, dconv), (1, gconv)):
                                w_off = (br * 9 + di * 3 + dj) * 64
                                rv = rhs_t[:, nh * 8:(nh + 1) * 8,
                                           di:di + 8, dj:dj + 8]
                                nc.tensor.matmul(
                                    psc[nh][br * 64:(br + 1) * 64, :],
                                    lhsT=convw[:, w_off:w_off + 64],
                                    rhs=rv,
                                    start=(si == 0), stop=(si == 8))

                # relu(conv + bias) -> dg_sb [128, 1024] fp16
                tail_on = "tail" not in drop
                dg_sb = wpool.tile([128, 1024], DT16, tag=f"dg_sb{par}")
                if tail_on:
                    for nh in range(2):
                        nc.scalar.activation(
                            dg_sb[:, nh * 512:(nh + 1) * 512],
                            psc[nh][:],
                            mybir.ActivationFunctionType.Relu,
                            bias=bias128[:],
                            scale=1.0)

                    # ---- fusion + classifier (per half for pipelining) ----
                    HB = B // 2
                    psum_out = []
                    for nh in range(2):
                        dgh = dg_sb[:, nh * 512:(nh + 1) * 512].rearrange(
                            "p (b f) -> p b f", b=HB)
                        s_red = wpool.tile([128, HB, 1], F32,
                                           tag=f"s_red{par}{nh}",
                                           name=f"s_red{par}{nh}")
                        nc.vector.reduce_sum(s_red[:], dgh,
                                             axis=mybir.AxisListType.X)
                        tk_red = []
                        for k in range(2):
                            tmpk = spool.tile([128, HB, 64], DT16, tag="tmpk")
                            cc = ccls[:, k * 64:(k + 1) * 64].unsqueeze(1)
                            nc.vector.tensor_tensor(
                                tmpk[:], dgh, cc.broadcast_to([128, HB, 64]), MUL)
                            tkr = wpool.tile([128, HB, 1], F32,
                                             tag=f"tkr{par}_{nh}_{k}",
                                             name=f"tkr{par}_{nh}_{k}")
                            nc.vector.reduce_sum(tkr[:], tmpk[:],
                                                 axis=mybir.AxisListType.X)
                            tk_red.append(tkr)
                        po = bankB[0:HB, 288 + 16 * nh:293 + 16 * nh]
                        for k in range(2):
                            nc.tensor.matmul(po[0:HB, 2 * k:2 * k + 2],
                                             lhsT=tk_red[k][:], rhs=sel[:],
                                             start=True, stop=True)
                        nc.tensor.matmul(po[0:HB, 4:5], lhsT=s_red[:],
                                         rhs=fusw[:], start=True, stop=True)
                        psum_out.append(po)

                if tail_on:
                    for nh in range(2):
                        osb = wpool.tile([8, 5], F32, tag=f"osb{par}{nh}",
                                         name=f"osb{par}{nh}")
                        nc.vector.tensor_copy(osb[:], psum_out[nh])
                        nc.scalar.dma_start(out_d[nh * 8:(nh + 1) * 8, :],
                                            osb[:])
                else:
                    out_sb = wpool.tile([16, 5], F32, tag=f"out_sb{par}")
                    nc.vector.memset(out_sb[:], 0.0)
                    nc.scalar.dma_start(out_d[:], out_sb[:])

            if timing_loop is None:
                emit(0)
            else:
                krep = int(_os.environ.get("KREP", "2"))
                kw = {}
                if _os.environ.get("KSTAG", "0") == "1":
                    kw["staggered_reset"] = True
                if _os.environ.get("KHINT", "0") == "1":
                    kw["hint_engines"] = (mybir.EngineType.PE,
                                          mybir.EngineType.SP,
                                          mybir.EngineType.DVE,
                                          mybir.EngineType.Activation)
                with tc.For_i(0, timing_loop, 1, **kw):
                    for r in range(krep):
                        emit(r % 2)

    nc.compile()
    return nc


_NC = {}


def _get_nc(timing_loop=None):
    if timing_loop not in _NC:
        _NC[timing_loop] = _build_nc(timing_loop)
    return _NC[timing_loop]


def _make_consts(conv_dct_w, conv_dct_b, bn_dct_g, bn_dct_b,
                 conv_grad_w, conv_grad_b, bn_grad_g, bn_grad_b,
                 fus_w, cls_w):
    A = build_A()
    AT = np.ascontiguousarray(A.T)  # [256, 8]
    consts = {}
    at = np.zeros((512, 8), np.float32)
    at[0:128] = AT[0::2]
    at[128:256] = AT[1::2]
    at[256:384] = AT[0:128]
    at[384:512] = AT[128:256]
    consts["at"] = at.astype(NP_DT16)

    BN_EPS = 1e-5
    convw = np.zeros((3, 1152), np.float32)
    bias = np.zeros((128, 1), np.float32)
    for br, (w, b, g, beta) in enumerate((
            (conv_dct_w, conv_dct_b, bn_dct_g, bn_dct_b),
            (conv_grad_w, conv_grad_b, bn_grad_g, bn_grad_b))):
        g_eff = (g / np.sqrt(1.0 + BN_EPS)).astype(np.float32)
        w_eff = w * g_eff[:, None, None, None]
        if br == 1:
            w_eff = w_eff * 0.25  # fold the 4-sample average
        b_eff = b * g_eff + beta
        for di in range(3):
            for dj in range(3):
                # convw[ic, (br*9+di*3+dj)*64 + oc] = w_eff[oc, ic, di, dj]
                off = (br * 9 + di * 3 + dj) * 64
                convw[:, off:off + 64] = w_eff[:, :, di, dj].T
        bias[br * 64:(br + 1) * 64, 0] = b_eff
    consts["convw"] = convw.astype(NP_DT16)
    consts["bias"] = bias

    ccls = np.zeros((128, 128), np.float32)
    for k in range(2):
        ccls[0:64, k * 64:(k + 1) * 64] = cls_w[k].reshape(64, 64)
        ccls[64:128, k * 64:(k + 1) * 64] = cls_w[k].reshape(64, 64)
    consts["ccls"] = ccls.astype(NP_DT16)
    consts["fusw"] = np.ascontiguousarray(np.tile(fus_w[0][:, None] / 64.0, (2, 1)))
    sel = np.zeros((128, 2), np.float32)
    sel[0:64, 0] = 1.0
    sel[64:128, 1] = 1.0
    consts["sel"] = sel
    return consts


def kernel_with_results(x, conv_dct_w, conv_dct_b, bn_dct_g, bn_dct_b,
                        conv_grad_w, conv_grad_b, bn_grad_g, bn_grad_b,
                        fus_w, fus_b, cls_w, cls_b, trace=False):
    nc = _get_nc()
    consts = _make_consts(conv_dct_w, conv_dct_b, bn_dct_g, bn_dct_b,
                          conv_grad_w, conv_grad_b, bn_grad_g, bn_grad_b,
                          fus_w, cls_w)
    x = np.ascontiguousarray(np.asarray(x, np.float32))
    in_maps = []
    for i in range(N_CORES):
        m = {"x": np.ascontiguousarray(x[i * B:(i + 1) * B])}
        m.update(consts)
        in_maps.append(m)
    res = run_bass_kernel_spmd(nc, in_maps, list(range(N_CORES)), trace=trace)

    outs = []
    for i in range(N_CORES):
        r = res.results[i]["out"]  # [16, 5]
        Pd = r[:, [0, 2]]
        Pg = r[:, [1, 3]]
        sl = r[:, 4] + np.float32(fus_b[0])
        w = 1.0 / (1.0 + np.exp(-sl))[:, None]
        outs.append(w * Pd + (1.0 - w) * Pg + np.asarray(cls_b)[None, :])
    return np.concatenate(outs, axis=0).astype(np.float32), res


def kernel(**inputs):
    out, _ = kernel_with_results(**inputs)
    return out
